# revision 19
# baseline (speedup 1.0000x reference)
"""Trainium2 Bass kernel v7 for nn_CLGF_GNNDrug (GCN+GIN drug GNN, 8 cores).

Key changes vs v2 baseline (3.18ms):
  - SWDGE dma_gather calls round-robin over 4 SWDGE queues: the single-queue
    ucode stalls on its own ring drain (7.6us/call); 4 queues pipeline prep
    against DMA drain (~2.0us/call, gathers run at ~230GB/s vs ~96GB/s).
  - Sorted-src chunking against HALF tables (2 x 51200 rows) addressed via
    offset-view in_aps (int16 idx is relative to a per-call base row), giving
    ~96% chunk fill vs 67% for the old (tile, quarter) cell scheme. Edge rows
    per layer drop from ~113k to ~80k.
  - Tables AllGathered in 2 chunks/layer (halves), overlapped with compute.
"""
import os
import sys
import types

import numpy as np
import ml_dtypes


def _install_ntff_hook():
    try:
        from antenv.axon_hooks import get_axon_ntff_profile_hook  # noqa: F401
        return
    except ImportError:
        pass
    try:
        from trn_agent_boot.trn_boot import _ntff_profile_via_ctypes
        hook = _ntff_profile_via_ctypes("/opt/axon/libaxon_pjrt.so")
    except Exception:
        hook = None
    mod = types.ModuleType("antenv.axon_hooks")
    mod.get_axon_ntff_profile_hook = lambda: hook
    mod.set_axon_ntff_profile_hook = lambda h: None
    sys.modules["antenv.axon_hooks"] = mod


_install_ntff_hook()

import concourse.bass as bass
import concourse.bacc as bacc
import concourse.mybir as mybir
import concourse.tile as tile
from concourse.bass_utils import run_bass_kernel_spmd

N = 100000
E = 500000
NG = 4000
F_IN = 77
D = 128
BN_EPS = 1e-5
NC = 8
P = 128
GRP = 512
GPT = GRP // P
NH = 2             # half tables per layer
NEG = -1.0e30
MAXCH = 8          # max chunks (x128 idxs) per dma_gather call
WIN = 32768        # int16 idx window (rows) per gather call
NQUEUE = 4         # SWDGE queues

dt = mybir.dt
BF = dt.float16
F32 = dt.float32
bf16 = np.float16


# ============================= host preprocessing =============================

def prep(x, edge_index, batch):
    x = np.asarray(x, np.float32)
    src_all = np.asarray(edge_index[0], np.int64)
    dst_all = np.asarray(edge_index[1], np.int64)
    batch = np.asarray(batch, np.int64)

    gsizes = np.bincount(batch, minlength=NG)
    gstart = np.concatenate([[0], np.cumsum(gsizes)])
    cuts = [0]
    for c in range(1, NC):
        target = c * N // NC
        g = int(np.searchsorted(gstart, target))
        if g > 0 and abs(gstart[g - 1] - target) < abs(gstart[min(g, NG)] - target):
            g -= 1
        g = min(max(g, cuts[-1]), NG)
        cuts.append(g)
    cuts.append(NG)
    g0 = np.array(cuts[:-1]); g1 = np.array(cuts[1:])
    n0 = gstart[g0]; n1 = gstart[g1]
    ncore = (n1 - n0).astype(np.int64)

    S = int(np.ceil(ncore.max() / GRP) * GRP)
    T = S // P
    NGRP = S // GRP
    HR = S // NH           # local rows per half
    NR = NC * HR           # rows per half-table
    Gc = (g1 - g0).astype(np.int64)
    G_pad = int(np.ceil((Gc.max() + 1) / 16) * 16)

    core_of = np.searchsorted(n1, np.arange(N), side="right")
    local = np.arange(N) - n0[core_of]
    shalf = local // HR                    # source half of each node
    srel = core_of * HR + (local % HR)     # row within half-table

    deg = 1.0 + np.bincount(dst_all, minlength=N).astype(np.float64)
    dinv = (1.0 / np.sqrt(deg)).astype(np.float32)

    # regular edges only — self-loops are handled via dedicated per-tile
    # "self chunks" whose rhs is a contiguous load from this core's own
    # emitted rows (ag buffers), not a gather.
    es = src_all
    ed = dst_all

    ecore = core_of[ed]
    dloc = local[ed]
    tl = dloc // P
    fc = (dloc % P).astype(np.int64)
    hh = shalf[es]
    rr = srel[es]

    # per-core edge lists sorted by (cell, table row)
    pc = {}
    for c in range(NC):
        idxs = np.where(ecore == c)[0]
        k_c = tl[idxs] * NH + hh[idxs]
        o = np.lexsort((rr[idxs], k_c))
        idxs = idxs[o]; k_c = k_c[o]
        cnts = np.bincount(k_c, minlength=T * NH)
        cb = np.concatenate([[0], np.cumsum(cnts)])[:-1]
        pc[c] = (idxs, k_c, cnts, cb)

    # shared (SPMD-uniform) segment cuts per cell: cut whenever any core
    # would exceed 128 rows in the segment, or the row span would exceed
    # the int16 gather window.
    nch = np.zeros((T, NH), np.int64)
    cell_cuts = {}
    for t in range(T):
        for h in range(NH):
            k = t * NH + h
            lists = [rr[pc[c][0][pc[c][3][k]:pc[c][3][k] + pc[c][2][k]]]
                     for c in range(NC)]
            ptr = [0] * NC
            cuts = []
            while True:
                rem = [lst[q:] for lst, q in zip(lists, ptr)]
                if all(len(r) == 0 for r in rem):
                    break
                first = min(int(r[0]) for r in rem if len(r))
                cut = first + WIN          # exclusive upper bound
                for r in rem:
                    if len(r) > P:
                        cut = min(cut, int(r[P]))
                assert cut > first, "degenerate segment (>128 equal rows)"
                cuts.append(cut)
                for ci, lst in enumerate(lists):
                    ptr[ci] += int(np.searchsorted(lst[ptr[ci]:], cut))
            cell_cuts[k] = np.array(cuts if cuts else [1], np.int64)
            nch[t, h] = len(cuts)
    MAXJ = max(int(nch.max()), 1)

    # per-core slot assignment + chunk row ranges
    cmin = np.full((T, NH, MAXJ), np.iinfo(np.int64).max, np.int64)
    cmax = np.full((T, NH, MAXJ), -1, np.int64)
    order_all = {}
    for c in range(NC):
        idxs, k_c, cnts, cb = pc[c]
        j = np.zeros(len(idxs), np.int64)
        for k in np.unique(k_c):
            sl = slice(cb[k], cb[k] + cnts[k])
            j[sl] = np.searchsorted(cell_cuts[k], rr[idxs[sl]], side="right")
        key2 = k_c * MAXJ + j
        gs = np.ones(len(idxs), bool)
        gs[1:] = key2[1:] != key2[:-1]
        startidx = np.maximum.accumulate(np.where(gs, np.arange(len(idxs)), 0))
        rank = np.arange(len(idxs)) - startidx
        assert len(rank) == 0 or rank.max() < P
        order_all[c] = (idxs, k_c, j, rank)
        np.minimum.at(cmin.reshape(-1), key2, rr[idxs])
        np.maximum.at(cmax.reshape(-1), key2, rr[idxs])
    empty = cmax.reshape(-1) < 0
    cmin.reshape(-1)[empty] = 0
    cmax.reshape(-1)[empty] = 0

    # call packing per (group, half): greedy by ascending min row.
    # Self chunks (one per tile, rhs loaded contiguously from ag buffers)
    # come first in each group.
    chunkpos = np.full((T, NH, MAXJ), -1, np.int64)
    basearr = []          # per chunk position: call base row
    band_of = []
    calls_by_g = [[] for _ in range(NGRP)]
    selfs_by_g = [[] for _ in range(NGRP)]
    selfpos = np.zeros(T, np.int64)
    pos = 0
    gs0 = []; gcnt = []
    for g in range(NGRP):
        gs0.append(pos)
        for t in range(g * GPT, (g + 1) * GPT):
            k_half = (t * P) // HR
            r0 = t * P - k_half * HR
            selfpos[t] = pos
            selfs_by_g[g].append((pos, t, k_half, r0))
            basearr.append(0)
            band_of.append(t % GPT)
            pos += 1
        for h in range(NH):
            chunks = []
            for t in range(g * GPT, (g + 1) * GPT):
                for j in range(int(nch[t, h])):
                    chunks.append((int(cmin[t, h, j]), int(cmax[t, h, j]), t, j))
            chunks.sort()
            cur = []
            cur_base = 0
            cur_max = 0

            def flush():
                nonlocal pos, cur
                if not cur:
                    return
                pos0 = pos
                for (mn, mx, t, j) in cur:
                    chunkpos[t, h, j] = pos
                    basearr.append(cur_base)
                    band_of.append(t % GPT)
                    pos += 1
                calls_by_g[g].append((h, pos0, len(cur), cur_base))
                cur = []

            for (mn, mx, t, j) in chunks:
                if cur and (len(cur) >= MAXCH or mx - cur_base > WIN - 1):
                    flush()
                if not cur:
                    cur_base = mn
                    cur_max = mx
                cur_max = max(cur_max, mx)
                cur.append((mn, mx, t, j))
                assert cur_max - cur_base <= WIN - 1
            flush()
        gcnt.append(pos - gs0[-1])
    C = pos
    basearr = np.array(basearr, np.int64)
    band_of = np.array(band_of, np.int64)

    # per-tile matmul order: self chunk first, then gather chunks
    tile_chunks = []
    for t in range(T):
        lst = [int(selfpos[t])] + sorted(
            int(p) for p in chunkpos[t].reshape(-1) if p >= 0)
        tile_chunks.append(lst)
    # per-group position count of selfs + half-0 calls (available before the
    # half-1 AllGather)
    n_h0 = []
    for g in range(NGRP):
        n = GPT + sum(nck for (h, p0, nck, b) in calls_by_g[g] if h == 0)
        n_h0.append(n)

    # per-core slot data
    xrow = np.zeros((N, 2 * P), bf16)
    xrow[:, :F_IN] = (dinv[:, None] * x).astype(bf16)
    xrow[:, P:P + F_IN] = x.astype(bf16)

    src16 = np.zeros((NC, C * P), np.int16)
    m = np.zeros((NC, P, C * P), bf16)
    xe = np.zeros((NC, P, C * 2 * P), bf16)
    for c in range(NC):
        idxs, k_c, j, rank = order_all[c]
        p = rank
        t_c = k_c // NH
        h_c = k_c % NH
        pos_e = chunkpos[t_c, h_c, j]
        assert (pos_e >= 0).all()
        rel = rr[idxs] - basearr[pos_e]
        if len(rel):
            assert rel.min() >= 0 and rel.max() <= WIN - 1
        src16[c, pos_e * P + p] = rel.astype(np.int16)
        m[c, p, pos_e * P + fc[idxs]] = 1.0
        xe[c].reshape(P, C, 2 * P)[p, pos_e, :] = xrow[es[idxs]]
        # self-loop chunks: node (t*P + p) at slot p of selfpos[t]
        nreal = int(ncore[c])
        loc = np.arange(nreal)
        tt = loc // P
        ps = loc % P
        spos = selfpos[tt]
        m[c, ps, spos * P + ps] = 1.0
        xe[c].reshape(P, C, 2 * P)[ps, spos, :] = xrow[n0[c] + loc]

    # wrapped int16 idx packing (idx n of chunk k -> [n%16, k*8 + n//16])
    idx16 = np.zeros((NC, P, C * 8), np.int16)
    for c in range(NC):
        w = src16[c].reshape(C * 8, 16).T
        idx16[c] = np.tile(w, (8, 1))

    resets = np.zeros((NC, S), np.float32)
    end_ids = np.zeros((NC, G_pad), np.int64)
    end_par = np.zeros((NC, G_pad), np.float32)
    for c in range(NC):
        gs = gstart[g0[c]:g1[c] + 1] - n0[c]
        starts = gs[:-1]; ends = gs[1:] - 1
        ne = gsizes[g0[c]:g1[c]] > 0
        resets[c, starts[ne]] = NEG
        if ncore[c] < S:
            resets[c, ncore[c]] = NEG
        end_ids[c, :g1[c] - g0[c]][ne] = ends[ne] // 2
        end_par[c, :g1[c] - g0[c]][ne] = (ends[ne] % 2).astype(np.float32)

    cntdeg = np.zeros((NC, S), np.float32)
    dinv_nm = np.ones((NC, P, T), np.float32)
    for c in range(NC):
        cntdeg[c, :ncore[c]] = deg[n0[c]:n1[c]].astype(np.float32)
        dv = dinv[n0[c]:n1[c]]
        dpad = np.ones(S, np.float32)
        dpad[:ncore[c]] = dv
        dinv_nm[c] = dpad.reshape(T, P).T

    return dict(
        S=S, T=T, NGRP=NGRP, HR=HR, NR=NR, C=C, G_pad=G_pad,
        calls_by_g=calls_by_g, selfs_by_g=selfs_by_g,
        gs0=gs0, gcnt=gcnt, band_of=band_of,
        tile_chunks=tile_chunks, n_h0=n_h0,
        g0=g0, g1=g1, n0=n0, n1=n1, ncore=ncore,
        src16=src16, m=m, xe=xe, idx16=idx16,
        resets=resets, end_ids=end_ids, end_par=end_par,
        empty=(gsizes == 0), cnt=cntdeg, dinv_nm=dinv_nm,
    )


# ============================= device program =============================

def build_program(meta, debug=False):
    S = meta["S"]; T = meta["T"]; NGRP = meta["NGRP"]
    HR = meta["HR"]; NR = meta["NR"]; C = meta["C"]; G_pad = meta["G_pad"]
    calls_by_g = meta["calls_by_g"]
    selfs_by_g = meta["selfs_by_g"]
    gs0 = meta["gs0"]; gcnt = meta["gcnt"]
    tile_chunks = meta["tile_chunks"]; n_h0 = meta["n_h0"]

    nc = bacc.Bacc("TRN2", target_bir_lowering=False, num_swdge_queues=NQUEUE)
    AluOp = mybir.AluOpType
    Act = mybir.ActivationFunctionType

    xe_d = nc.dram_tensor("xe", [P, C * 2 * P], BF, kind="ExternalInput")
    m_d = nc.dram_tensor("m", [P, C * P], BF, kind="ExternalInput")
    idx_d = nc.dram_tensor("idx16", [P, C * 8], dt.int16, kind="ExternalInput")
    wstk = nc.dram_tensor("wstk", [8 * P, D], BF, kind="ExternalInput")
    pvec = nc.dram_tensor("pvec", [P, 14], F32, kind="ExternalInput")
    rst_d = nc.dram_tensor("resets", [P, S], BF, kind="ExternalInput")
    endi = nc.dram_tensor("endi", [P, G_pad // 16], dt.int16, kind="ExternalInput")
    bncor = nc.dram_tensor("bncor", [P, 6], F32, kind="ExternalInput")
    ident = nc.dram_tensor("ident", [P, P], BF, kind="ExternalInput")
    identf = nc.dram_tensor("identf", [P, P], F32, kind="ExternalInput")
    cntv = nc.dram_tensor("cntv", [1, S], BF, kind="ExternalInput")
    dinv_d = nc.dram_tensor("dinv", [P, T], F32, kind="ExternalInput")
    parw_d = nc.dram_tensor("parw", [P, G_pad], F32, kind="ExternalInput")

    out = nc.dram_tensor("out", [G_pad * 9, D], F32, kind="ExternalOutput")

    ag2h = [nc.dram_tensor(f"ag2_{k}", [HR, 2 * D], BF) for k in range(NH)]
    tab2h = [nc.dram_tensor(f"tab2_{k}", [NR, 2 * D], BF, addr_space="Shared")
             for k in range(NH)]
    ag3h = [nc.dram_tensor(f"ag3_{k}", [HR, D], BF) for k in range(NH)]
    tab3h = [nc.dram_tensor(f"tab3_{k}", [NR, D], BF, addr_space="Shared")
             for k in range(NH)]
    bn_in = [nc.dram_tensor(f"bn{i}_in", [P, 2], F32) for i in range(3)]
    bn_out = [nc.dram_tensor(f"bn{i}_out", [P, 2], F32, addr_space="Shared")
              for i in range(3)]
    SL = {}
    slkind = dict(kind="ExternalOutput") if debug else {}
    for nme in ("xg1", "xg2", "u0", "u1", "u2"):
        SL[nme] = nc.dram_tensor(f"sl_{nme}", [P, S], BF, **slkind)
    if debug:
        dbg_scan = nc.dram_tensor("dbg_scan", [P, 2 * S], BF,
                                  kind="ExternalOutput")
        dbg_ext = nc.dram_tensor("dbg_ext", [P, 4 * G_pad], BF,
                                 kind="ExternalOutput")
        dbg_pool = nc.dram_tensor("dbg_pool", [P, G_pad * 9], F32,
                                  kind="ExternalOutput")

    RG = [list(range(NC))]
    # AllGather chunk trigger group: half k ready after group trig[k]
    trig = {}
    for k in range(NH):
        trig[((k + 1) * HR - 1) // GRP] = k

    qctr = [0]

    def next_q():
        q = qctr[0] % NQUEUE
        qctr[0] += 1
        return q

    with tile.TileContext(nc) as tc:
        with (
            tc.tile_pool(name="cst", bufs=1) as cst,
            tc.tile_pool(name="scn", bufs=3) as scnp,
            tc.tile_pool(name="gat", bufs=2) as gat,
            tc.tile_pool(name="mbuf", bufs=2) as mbp,
            tc.tile_pool(name="work", bufs=2) as wkp,
            tc.tile_pool(name="one", bufs=1) as onep,
            tc.tile_pool(name="ps2b", bufs=4, space="PSUM") as psa_p,
            tc.tile_pool(name="psd", bufs=1, space="PSUM") as psd,
            tc.tile_pool(name="pst", bufs=2, space="PSUM") as pst,
        ):
            # ---------------- constants ----------------
            w_sb = cst.tile([P, 8 * D], BF)
            for i in range(8):
                nc.sync.dma_start(out=w_sb[:, i * D:(i + 1) * D],
                                  in_=wstk[i * P:(i + 1) * P, :])
            pv = cst.tile([P, 14], F32)
            nc.sync.dma_start(out=pv[:], in_=pvec[:, :])
            endi_sb = cst.tile([P, G_pad // 16], dt.int16)
            nc.sync.dma_start(out=endi_sb[:], in_=endi[:, :])
            idn = cst.tile([P, P], BF)
            nc.sync.dma_start(out=idn[:], in_=ident[:, :])
            idnf = cst.tile([P, P], F32)
            nc.sync.dma_start(out=idnf[:], in_=identf[:, :])
            bnc = cst.tile([P, 6], F32)
            nc.sync.dma_start(out=bnc[:], in_=bncor[:, :])
            dinv_sb = cst.tile([P, T], F32)
            nc.sync.dma_start(out=dinv_sb[:], in_=dinv_d[:, :])
            parw_sb = cst.tile([P, G_pad], F32)
            nc.sync.dma_start(out=parw_sb[:], in_=parw_d[:, :])
            aeff_sb = cst.tile([P, 2 * D], BF)
            atv_row = cst.tile([1, 2 * P], BF)

            W1, W2 = w_sb[:, 0:D], w_sb[:, D:2 * D]
            A = [w_sb[:, (2 + 2 * i) * D:(3 + 2 * i) * D] for i in range(3)]
            B = [w_sb[:, (3 + 2 * i) * D:(4 + 2 * i) * D] for i in range(3)]
            b1c, b2c = pv[:, 0:1], pv[:, 1:2]
            ac = [pv[:, 2 + 4 * i:3 + 4 * i] for i in range(3)]
            cc = [pv[:, 3 + 4 * i:4 + 4 * i] for i in range(3)]
            gcl = [pv[:, 4 + 4 * i:5 + 4 * i] for i in range(3)]
            bec = [pv[:, 5 + 4 * i:6 + 4 * i] for i in range(3)]

            pooled = cst.tile([P, G_pad * 9], F32)
            stats = cst.tile([P, 3 * 2 * NGRP], F32)
            sf = cst.tile([P, 8], F32)
            bnreg = cst.tile([P, 9], F32)

            scanbufs = {}
            prevs = {}

            def load_rst(g):
                rt = wkp.tile([P, GRP], BF, tag="rstg", name=f"rst_{g}")
                nc.sync.dma_start(out=rt[:],
                                  in_=rst_d[:, g * GRP:(g + 1) * GRP])
                return rt

            def scan_g(comp, g, data1, rt):
                if comp not in scanbufs:
                    scanbufs[comp] = scnp.tile([P, S], BF, tag="scan",
                                               name=f"scan_c{comp}")
                    prevs[comp] = 0.0
                sb = scanbufs[comp]
                nc.vector.tensor_tensor_scan(
                    out=sb[:, g * GRP:(g + 1) * GRP],
                    data0=rt[:],
                    data1=data1,
                    initial=prevs[comp],
                    op0=AluOp.add,
                    op1=AluOp.max)
                prevs[comp] = sb[:, (g + 1) * GRP - 1:(g + 1) * GRP]

            def extract(comp):
                # bf16 ap_gather needs d=2: gather the PAIR containing the
                # graph-end column, then select the half via the parity mask.
                sb = scanbufs[comp]
                ext = wkp.tile([P, G_pad * 2], BF, tag="nm",
                                name=f"ext{comp}")
                nc.gpsimd.ap_gather(
                    out_ap=ext[:].rearrange("p (g o) -> p g o", o=2),
                    in_ap=sb[:].rearrange("p (s o) -> p s o", o=2),
                    idxs_ap=endi_sb[:], channels=P, num_elems=S // 2, d=2,
                    num_idxs=G_pad)
                pv_ = pooled[:, comp::9]
                nc.vector.tensor_tensor(out=pv_, in0=ext[:, 1::2],
                                        in1=ext[:, 0::2], op=AluOp.subtract)
                nc.vector.tensor_tensor(out=pv_, in0=pv_, in1=parw_sb[:],
                                        op=AluOp.mult)
                nc.vector.tensor_tensor(out=pv_, in0=pv_, in1=ext[:, 0::2],
                                        op=AluOp.add)
                if debug and comp in (0, 4):
                    hf = comp // 4
                    nc.sync.dma_start(out=dbg_scan[:, hf * S:(hf + 1) * S],
                                      in_=sb[:])
                    nc.sync.dma_start(
                        out=dbg_ext[:, hf * 2 * G_pad:(hf + 1) * 2 * G_pad],
                        in_=ext[:])

            # PSUM bands live at 2KB (bank) strides: a matmul with start=True
            # zeroes its entire PSUM bank, so two open accumulations must
            # never share a bank.
            BANDW = 512  # f32 elems per band slot = one 2KB bank

            def agg_tiles(g, rhs0, rhs1, split, elem, halves):
                """Per-tile band accumulation -> feature-major agg tiles.

                Phase A issues every tile's half-0 (selfs + early-table)
                matmuls first, so the PE keeps working while the half-1
                AllGather is still in flight; phase A2 adds the half-1
                matmuls; phase B drains the bands (copies + transposes).
                Bands rotate through 4 PSUM banks."""
                s0 = gs0[g]
                aggs = {}
                for nme, _ in halves:
                    aggs[nme] = wkp.tile([P, GRP], BF, tag=f"agg{nme}",
                                         name=f"agg_{nme}_{g}")
                bands = []
                for ti in range(GPT):
                    t = g * GPT + ti
                    lst = tile_chunks[t]
                    lst0 = [p for p in lst if p - s0 < split]
                    lst1 = [p for p in lst if p - s0 >= split]
                    band = psa_p.tile([P, BANDW], F32, tag="psa",
                                      name=f"psa_{elem}_{g}_{ti}")
                    bands.append((band, lst0, lst1))
                    for i, pos in enumerate(lst0):
                        off = pos - s0
                        nc.tensor.matmul(
                            band[:, 0:elem],
                            m_t[:, off * P:(off + 1) * P],
                            rhs0[:, off * elem:(off + 1) * elem],
                            start=(i == 0),
                            stop=(not lst1 and i == len(lst0) - 1))
                for ti in range(GPT):
                    band, lst0, lst1 = bands[ti]
                    for i, pos in enumerate(lst1):
                        off = pos - s0
                        o2 = off - split
                        nc.tensor.matmul(
                            band[:, 0:elem],
                            m_t[:, off * P:(off + 1) * P],
                            rhs1[:, o2 * elem:(o2 + 1) * elem],
                            start=False, stop=(i == len(lst1) - 1))
                for ti in range(GPT):
                    t = g * GPT + ti
                    band = bands[ti][0]
                    col = dinv_sb[:, t:t + 1]
                    nm = wkp.tile([P, len(halves) * P], BF, tag="nm",
                                  name=f"nm_{elem}_{g}_{ti}")
                    for hi, (nme, scaled) in enumerate(halves):
                        src = band[:, hi * P:(hi + 1) * P]
                        dst = nm[:, hi * P:(hi + 1) * P]
                        if scaled:
                            nc.scalar.activation(dst, src, Act.Identity,
                                                 scale=col)
                        elif hi % 2 == 0:
                            nc.scalar.copy(dst, src)
                        else:
                            nc.vector.tensor_copy(dst, src)
                    for hi, (nme, _) in enumerate(halves):
                        ptile = pst.tile([P, P], BF, tag="tp",
                                         name=f"tp_{nme}_{g}_{ti}")
                        nc.tensor.transpose(
                            ptile[:], nm[:, hi * P:(hi + 1) * P], idn[:])
                        if ti % 2 == 0:
                            nc.vector.tensor_copy(
                                aggs[nme][:, ti * P:(ti + 1) * P], ptile[:])
                        else:
                            nc.scalar.copy(
                                aggs[nme][:, ti * P:(ti + 1) * P], ptile[:])
                return aggs

            def emit_nm(g, o_tile, dest_list, col0, width, scale_dinv):
                """Transpose feature-major dense output to node-major rows and
                DMA into the collective-input tensors (split at HR bounds)."""
                em = wkp.tile([P, GPT * P], BF, tag="em", name=f"em_{g}_{col0}")
                for t in range(GPT):
                    ptile = pst.tile([P, P], BF, tag="tp",
                                     name=f"em_tp_{g}_{col0}_{t}")
                    nc.tensor.transpose(ptile[:], o_tile[:, t * P:(t + 1) * P],
                                        idn[:])
                    if scale_dinv:
                        nc.scalar.activation(
                            em[:, t * P:(t + 1) * P], ptile[:], Act.Identity,
                            scale=dinv_sb[:, g * GPT + t:g * GPT + t + 1])
                    else:
                        nc.scalar.copy(em[:, t * P:(t + 1) * P], ptile[:])
                # DMA node-major: rows g*GRP + t*128 + p
                r0 = g * GRP
                t0 = 0
                while t0 < GPT:
                    k = (r0 + t0 * P) // HR
                    tmax = min(GPT, ((k + 1) * HR - r0) // P)
                    nt = tmax - t0
                    dest = dest_list[k]
                    rr_ = r0 + t0 * P - k * HR
                    dst_ap = dest[rr_:rr_ + nt * P, col0:col0 + width]
                    dst_ap = dst_ap.rearrange("(t p) f -> p t f", p=P)
                    src_ap = em[:, t0 * P:(t0 + nt) * P]
                    src_ap = src_ap.rearrange("p (t f) -> p t f", f=P)
                    nc.sync.dma_start(out=dst_ap, in_=src_ap)
                    t0 = tmax

            def dense_gcn(g, aggbuf, W, bcol, sl_dest):
                ps = psd.tile([P, GRP], F32, tag="zd")
                nc.tensor.matmul(ps[:], W, aggbuf[:], start=True, stop=True)
                o = wkp.tile([P, GRP], BF, tag="obf", name=f"ogcn_{g}")
                nc.scalar.activation(o[:], ps[:], Act.Relu, bias=bcol)
                nc.sync.dma_start(
                    out=sl_dest[:, g * GRP:(g + 1) * GRP], in_=o[:])
                return o

            def dense_gin(g, aggbuf, li, sl_dest):
                ps1 = psd.tile([P, GRP], F32, tag="zd")
                if li == 0:
                    nc.tensor.matmul(ps1[:], A[0], aggbuf[:],
                                     start=True, stop=True)
                else:
                    Aeff = aeff_sb[:, (li - 1) * D:li * D]
                    nc.tensor.matmul(ps1[:], Aeff, aggbuf[:],
                                     start=True, stop=False)
                    cg = wkp.tile([1, GRP], BF, tag="cntg",
                                  name=f"cg{li}_{g}")
                    nc.sync.dma_start(
                        out=cg[:], in_=cntv[0:1, g * GRP:(g + 1) * GRP])
                    nc.tensor.matmul(
                        ps1[:], atv_row[0:1, (li - 1) * P:li * P],
                        cg[0:1, :], start=False, stop=True)
                ua = wkp.tile([P, GRP], BF, tag="ua", name=f"ua{li}_{g}")
                nc.scalar.activation(ua[:], ps1[:], Act.Relu, bias=ac[li])
                ps2 = psd.tile([P, GRP], F32, tag="zd2")
                nc.tensor.matmul(ps2[:], B[li], ua[:], start=True, stop=True)
                o32 = wkp.tile([P, GRP], F32, tag="o32", name=f"og{li}_{g}")
                base = li * 2 * NGRP
                nc.scalar.activation(o32[:], ps2[:], Act.Relu, bias=cc[li],
                                     accum_out=stats[:, base + 2 * g:
                                                     base + 2 * g + 1])
                sq = onep.tile([P, GRP], F32, tag="sq", name=f"sq{li}_{g}")
                nc.vector.tensor_tensor(out=sq[:], in0=o32[:], in1=o32[:],
                                        op=AluOp.mult)
                nc.vector.reduce_sum(
                    stats[:, base + 2 * g + 1:base + 2 * g + 2], sq[:],
                    axis=mybir.AxisListType.X)
                obf = wkp.tile([P, GRP], BF, tag="obf", name=f"ogb{li}_{g}")
                nc.vector.tensor_copy(obf[:], o32[:])
                nc.sync.dma_start(
                    out=sl_dest[:, g * GRP:(g + 1) * GRP], in_=obf[:])
                return o32, obf

            def bn_finalize(li):
                base = li * 2 * NGRP
                nc.vector.reduce_sum(sf[:, 0:1], stats[:, base:base + 2 * NGRP:2],
                                     axis=mybir.AxisListType.X)
                nc.vector.reduce_sum(sf[:, 1:2],
                                     stats[:, base + 1:base + 2 * NGRP:2],
                                     axis=mybir.AxisListType.X)
                nc.vector.tensor_tensor(out=sf[:, 0:2], in0=sf[:, 0:2],
                                        in1=bnc[:, 2 * li:2 * li + 2],
                                        op=AluOp.subtract)
                nc.sync.dma_start(out=bn_in[li][:, :], in_=sf[:, 0:2])
                nc.gpsimd.collective_compute(
                    "AllReduce", AluOp.add, replica_groups=RG,
                    ins=[bn_in[li][:, :]], outs=[bn_out[li][:, :]])
                nc.sync.dma_start(out=sf[:, 2:4], in_=bn_out[li][:, :])
                nc.vector.tensor_scalar(out=sf[:, 4:5], in0=sf[:, 2:3],
                                        scalar1=1.0 / N, scalar2=None,
                                        op0=AluOp.mult)
                nc.vector.tensor_scalar(out=sf[:, 5:6], in0=sf[:, 3:4],
                                        scalar1=1.0 / N, scalar2=None,
                                        op0=AluOp.mult)
                nc.vector.tensor_tensor(out=sf[:, 6:7], in0=sf[:, 4:5],
                                        in1=sf[:, 4:5], op=AluOp.mult)
                nc.vector.tensor_tensor(out=sf[:, 5:6], in0=sf[:, 5:6],
                                        in1=sf[:, 6:7], op=AluOp.subtract)
                nc.vector.tensor_scalar(out=sf[:, 5:6], in0=sf[:, 5:6],
                                        scalar1=BN_EPS, scalar2=None,
                                        op0=AluOp.add)
                nc.scalar.activation(sf[:, 5:6], sf[:, 5:6], Act.Sqrt)
                nc.vector.reciprocal(sf[:, 6:7], sf[:, 5:6])
                nc.vector.tensor_tensor(out=sf[:, 6:7], in0=sf[:, 6:7],
                                        in1=gcl[li], op=AluOp.mult)
                nc.vector.tensor_tensor(out=sf[:, 7:8], in0=sf[:, 4:5],
                                        in1=sf[:, 6:7], op=AluOp.mult)
                nc.vector.tensor_tensor(out=sf[:, 7:8], in0=bec[li],
                                        in1=sf[:, 7:8], op=AluOp.subtract)
                scol = bnreg[:, 3 * li:3 * li + 1]
                tcol = bnreg[:, 3 * li + 1:3 * li + 2]
                rcol = bnreg[:, 3 * li + 2:3 * li + 3]
                nc.vector.tensor_copy(scol, sf[:, 6:7])
                nc.vector.tensor_copy(tcol, sf[:, 7:8])
                nc.vector.reciprocal(sf[:, 0:1], sf[:, 6:7])
                nc.vector.tensor_tensor(out=rcol, in0=sf[:, 7:8],
                                        in1=sf[:, 0:1], op=AluOp.mult)
                if li < 2:
                    nc.vector.tensor_scalar(
                        out=aeff_sb[:, li * D:(li + 1) * D], in0=A[li + 1],
                        scalar1=sf[:, 6:7], scalar2=None, op0=AluOp.mult)
                    tbf = wkp.tile([P, 1], BF, tag="nm1", name=f"tbf{li}")
                    nc.vector.tensor_copy(tbf[:], sf[:, 7:8])
                    pv1 = pst.tile([P, 1], F32, tag="tp", name=f"atv{li}")
                    nc.tensor.matmul(pv1[:], A[li + 1], tbf[:],
                                     start=True, stop=True)
                    atc = wkp.tile([P, 1], BF, tag="nm1", name=f"atc{li}")
                    nc.vector.tensor_copy(atc[:], pv1[:])
                    pv2 = pst.tile([1, P], BF, tag="tp", name=f"atr{li}")
                    nc.tensor.transpose(pv2[:], atc[:], idn[:])
                    nc.vector.tensor_copy(atv_row[0:1, li * P:(li + 1) * P],
                                          pv2[:])

            def issue_h0(g, ag_list, tabs, elem):
                split = n_h0[g]
                gt0 = gat.tile([P, split * elem], BF, tag="g0",
                               name=f"g0{elem}_{g}")
                s0 = gs0[g]
                # self chunks: contiguous load of this core's own emitted rows
                for (pos, t, kh, r0) in selfs_by_g[g]:
                    off = pos - s0
                    nc.sync.dma_start(
                        out=gt0[:, off * elem:(off + 1) * elem],
                        in_=ag_list[kh][r0:r0 + P, :])
                idx_t = wkp.tile([P, gcnt[g] * 8], dt.int16, tag="idxg2",
                                 name=f"idx_{elem}_{g}")
                nc.sync.dma_start(
                    out=idx_t[:], in_=idx_d[:, s0 * 8:(s0 + gcnt[g]) * 8])
                for (h, pos0, nck, cbase) in calls_by_g[g]:
                    if h != 0:
                        continue
                    o0 = pos0 - s0
                    n = nck * P
                    W = min(WIN, NR - cbase)
                    nview = gt0[:, o0 * elem:(o0 + nck) * elem]
                    nc.gpsimd.dma_gather(
                        nview.rearrange("p (j e) -> p j e", e=elem),
                        tabs[0][cbase:cbase + W, :],
                        idx_t[:, o0 * 8:(o0 + nck) * 8],
                        n, n, elem, queue_num=next_q(),
                    )
                return gt0, idx_t

            def issue_h1(g, tabs, elem, idx_t):
                split = n_h0[g]
                gt1 = gat.tile([P, (gcnt[g] - split) * elem], BF, tag="g1",
                               name=f"g1{elem}_{g}")
                s0 = gs0[g]
                for (h, pos0, nck, cbase) in calls_by_g[g]:
                    if h != 1:
                        continue
                    o0 = pos0 - s0
                    o1 = o0 - split
                    n = nck * P
                    W = min(WIN, NR - cbase)
                    nview = gt1[:, o1 * elem:(o1 + nck) * elem]
                    nc.gpsimd.dma_gather(
                        nview.rearrange("p (j e) -> p j e", e=elem),
                        tabs[1][cbase:cbase + W, :],
                        idx_t[:, o0 * 8:(o0 + nck) * 8],
                        n, n, elem, queue_num=next_q(),
                    )
                return gt1

            def load_m(g):
                mt = mbp.tile([P, gcnt[g] * P], BF, tag="m", name=f"m_{g}")
                nc.sync.dma_start(
                    out=mt[:], in_=m_d[:, gs0[g] * P:(gs0[g] + gcnt[g]) * P])
                return mt

            def load_sl(nme, g, tag):
                st = wkp.tile([P, GRP], BF, tag=tag, name=f"ld_{nme}_{g}")
                nc.sync.dma_start(out=st[:],
                                  in_=SL[nme][:, g * GRP:(g + 1) * GRP])
                return st

            # ================= LAYER 1 =================
            for g in range(NGRP):
                xe_t = gat.tile([P, gcnt[g] * 2 * P], BF, tag="g0",
                                name=f"xe_{g}")
                nc.sync.dma_start(
                    out=xe_t[:],
                    in_=xe_d[:, gs0[g] * 2 * P:(gs0[g] + gcnt[g]) * 2 * P])
                m_t = load_m(g)
                aggs = agg_tiles(g, xe_t, xe_t, gcnt[g], 2 * P,
                                 [("gc", True), ("gi", False)])
                rt = load_rst(g)
                o_xg1 = dense_gcn(g, aggs["gc"], W1, b1c, SL["xg1"])
                scan_g(0, g, o_xg1[:], rt)
                emit_nm(g, o_xg1, ag2h, 0, D, True)
                o_u0, o_u0b = dense_gin(g, aggs["gi"], 0, SL["u0"])
                scan_g(4, g, o_u0[:], rt)
                emit_nm(g, o_u0b, ag2h, D, D, False)
                if g in trig and trig[g] < NH - 1:
                    k = trig[g]
                    nc.gpsimd.collective_compute(
                        "AllGather", AluOp.bypass, replica_groups=RG,
                        ins=[ag2h[k][:, :]], outs=[tab2h[k][:, :]])

            nc.gpsimd.collective_compute(
                "AllGather", AluOp.bypass, replica_groups=RG,
                ins=[ag2h[NH - 1][:, :]], outs=[tab2h[NH - 1][:, :]])
            # seed two groups of half-0 gathers before the (blocking)
            # AllReduce in bn_finalize so the gpsimd stream keeps moving
            pend2 = {0: issue_h0(0, ag2h, tab2h, 2 * P),
                     1: issue_h0(1, ag2h, tab2h, 2 * P)}
            bn_finalize(0)
            extract(0)
            extract(4)

            # ================= LAYER 2 =================
            for g in range(NGRP):
                if g + 1 < NGRP and g + 1 not in pend2:
                    pend2[g + 1] = issue_h0(g + 1, ag2h, tab2h, 2 * P)
                gt0, idx_t = pend2.pop(g)
                gt1 = issue_h1(g, tab2h, 2 * P, idx_t)
                m_t = load_m(g)
                aggs = agg_tiles(g, gt0, gt1, n_h0[g], 2 * P,
                                 [("gc", True), ("gi", False)])
                rt = load_rst(g)
                o_xg2 = dense_gcn(g, aggs["gc"], W2, b2c, SL["xg2"])
                scan_g(1, g, o_xg2[:], rt)
                o_u1, o_u1b = dense_gin(g, aggs["gi"], 1, SL["u1"])
                scan_g(5, g, o_u1[:], rt)
                emit_nm(g, o_u1b, ag3h, 0, D, False)
                if g in trig and trig[g] < NH - 1:
                    k = trig[g]
                    nc.gpsimd.collective_compute(
                        "AllGather", AluOp.bypass, replica_groups=RG,
                        ins=[ag3h[k][:, :]], outs=[tab3h[k][:, :]])

            nc.gpsimd.collective_compute(
                "AllGather", AluOp.bypass, replica_groups=RG,
                ins=[ag3h[NH - 1][:, :]], outs=[tab3h[NH - 1][:, :]])
            pend3 = {0: issue_h0(0, ag3h, tab3h, P),
                     1: issue_h0(1, ag3h, tab3h, P)}
            bn_finalize(1)
            extract(1)
            extract(5)

            s1c = bnreg[:, 0:1]; t1c = bnreg[:, 1:2]; r1c = bnreg[:, 2:3]
            s2c = bnreg[:, 3:4]; t2c = bnreg[:, 4:5]; r2c = bnreg[:, 5:6]
            s3c = bnreg[:, 6:7]; t3c = bnreg[:, 7:8]; r3c = bnreg[:, 8:9]

            # comps 2/3 scans: only need xg1/xg2 — run while tab3h gathers
            # wait on the half-1 AllGather
            for g in range(NGRP):
                rt = load_rst(g)
                xg1t = load_sl("xg1", g, "ldA")
                xg2t = load_sl("xg2", g, "ldB")
                tsum = wkp.tile([P, GRP], BF, tag="tt1", name=f"c2in_{g}")
                nc.vector.tensor_tensor(out=tsum[:], in0=xg1t[:], in1=xg2t[:],
                                        op=AluOp.add)
                scan_g(2, g, tsum[:], rt)
                tprd = wkp.tile([P, GRP], BF, tag="tt2", name=f"c3in_{g}")
                nc.vector.tensor_tensor(out=tprd[:], in0=xg1t[:], in1=xg2t[:],
                                        op=AluOp.mult)
                scan_g(3, g, tprd[:], rt)

            # ================= LAYER 3 =================
            for g in range(NGRP):
                if g + 1 < NGRP and g + 1 not in pend3:
                    pend3[g + 1] = issue_h0(g + 1, ag3h, tab3h, P)
                gt0, idx_t = pend3.pop(g)
                gt1 = issue_h1(g, tab3h, P, idx_t)
                m_t = load_m(g)
                aggs = agg_tiles(g, gt0, gt1, n_h0[g], P, [("gi", False)])
                rt = load_rst(g)
                o_u2, _ = dense_gin(g, aggs["gi"], 2, SL["u2"])
                scan_g(6, g, o_u2[:], rt)

            bn_finalize(2)
            extract(2)
            extract(3)
            extract(6)

            # ================= TAIL: comps 7, 8 =================
            for g in range(NGRP):
                u0t = wkp.tile([P, GRP], BF, tag="ldA", name=f"lu0t_{g}")
                nc.sync.dma_start(out=u0t[:],
                                  in_=SL["u0"][:, g * GRP:(g + 1) * GRP])
                u1t = wkp.tile([P, GRP], BF, tag="ldB", name=f"lu1t_{g}")
                nc.sync.dma_start(out=u1t[:],
                                  in_=SL["u1"][:, g * GRP:(g + 1) * GRP])
                u2t = wkp.tile([P, GRP], BF, tag="ldC", name=f"lu2_{g}")
                nc.sync.dma_start(out=u2t[:],
                                  in_=SL["u2"][:, g * GRP:(g + 1) * GRP])
                rt = load_rst(g)
                h1 = wkp.tile([P, GRP], BF, tag="tt3", name=f"h1_{g}")
                nc.scalar.activation(h1[:], u0t[:], Act.Identity, bias=t1c,
                                     scale=s1c)
                h2 = wkp.tile([P, GRP], BF, tag="tt4", name=f"h2_{g}")
                nc.scalar.activation(h2[:], u1t[:], Act.Identity, bias=t2c,
                                     scale=s2c)
                h3 = wkp.tile([P, GRP], BF, tag="tt5", name=f"h3_{g}")
                nc.scalar.activation(h3[:], u2t[:], Act.Identity, bias=t3c,
                                     scale=s3c)
                z3 = wkp.tile([P, GRP], BF, tag="tt6", name=f"z3_{g}")
                nc.gpsimd.tensor_tensor(out=z3[:], in0=h1[:], in1=h2[:],
                                        op=AluOp.add)
                nc.vector.tensor_tensor(out=z3[:], in0=z3[:], in1=h3[:],
                                        op=AluOp.add)
                scan_g(7, g, z3[:], rt)
                w3 = wkp.tile([P, GRP], BF, tag="tt7", name=f"w3_{g}")
                nc.gpsimd.tensor_tensor(out=w3[:], in0=h1[:], in1=h2[:],
                                        op=AluOp.mult)
                nc.gpsimd.tensor_tensor(out=w3[:], in0=w3[:], in1=h3[:],
                                        op=AluOp.mult)
                scan_g(8, g, w3[:], rt)

            extract(7)
            extract(8)

            # pooled-domain BN affine fixups
            for comp, sc, tc_ in ((4, s1c, t1c), (5, s2c, t2c), (6, s3c, t3c)):
                nc.scalar.activation(pooled[:, comp::9], pooled[:, comp::9],
                                     Act.Identity, bias=tc_, scale=sc)

            if debug:
                nc.sync.dma_start(out=dbg_pool[:, :], in_=pooled[:])

            # final transpose-out
            NPT = (G_pad * 9 + P - 1) // P
            for t in range(NPT):
                c0 = t * P
                w = min(P, G_pad * 9 - c0)
                ptile = pst.tile([P, P], F32, tag="tp", name=f"po_{t}")
                nc.tensor.transpose(ptile[:w, :], pooled[:, c0:c0 + w], idnf[:])
                nmo = wkp.tile([P, P], F32, tag="obf", name=f"pon_{t}")
                nc.vector.tensor_copy(nmo[:w, :], ptile[:w, :])
                nc.sync.dma_start(out=out[c0:c0 + w, :], in_=nmo[:w, :])

    nc.finalize()
    return nc


# ============================= top-level kernel =============================

_CACHE = {}


def kernel(x, edge_index, batch, W1, b1, W2, b2,
           A0, a0, B0, c0, g0, be0,
           A1, a1, B1, c1, g1, be1,
           A2, a2, B2, c2, g2, be2):
    pp = prep(x, edge_index, batch)

    debug = bool(os.environ.get("KERNEL_DEBUG"))
    key = (pp["S"], pp["C"], pp["G_pad"], debug,
           repr(pp["calls_by_g"]), pp["band_of"].tobytes())
    if key not in _CACHE:
        _CACHE[key] = build_program(pp, debug=debug)
    nc = _CACHE[key]

    def pad_w(W):
        Wp = np.zeros((P, D), np.float32)
        W = np.asarray(W, np.float32)
        Wp[:W.shape[0]] = W
        return Wp

    wstk = np.concatenate([pad_w(W1), pad_w(W2), pad_w(A0), pad_w(B0),
                           pad_w(A1), pad_w(B1), pad_w(A2), pad_w(B2)],
                          axis=0).astype(bf16)
    pvec = np.stack([np.asarray(v, np.float32) for v in
                     (b1, b2, a0, c0, g0, be0, a1, c1, g1, be1,
                      a2, c2, g2, be2)], axis=1)
    ident = np.eye(P, dtype=bf16)
    identf = np.eye(P, dtype=np.float32)

    total_pads = NC * pp["S"] - N
    bncor = np.zeros((P, 6), np.float32)
    for li, (Aw, av, Bw, cv) in enumerate(
            ((A0, a0, B0, c0), (A1, a1, B1, c1), (A2, a2, B2, c2))):
        ua = np.maximum(np.asarray(av, np.float32), 0.0)
        u_pad = np.maximum(ua @ np.asarray(Bw, np.float32)
                           + np.asarray(cv, np.float32), 0.0)
        bncor[:, 2 * li] = total_pads * u_pad
        bncor[:, 2 * li + 1] = total_pads * u_pad * u_pad

    G_pad = pp["G_pad"]
    in_maps = []
    for c in range(NC):
        ends = pp["end_ids"][c].astype(np.int16)
        endw = ends.reshape(G_pad // 16, 16).T.copy()
        endw = np.tile(endw, (8, 1))
        parw = np.tile(pp["end_par"][c][None, :], (P, 1)).astype(np.float32)
        in_maps.append(dict(
            parw=parw,
            xe=pp["xe"][c],
            m=pp["m"][c],
            idx16=pp["idx16"][c],
            wstk=wstk,
            pvec=pvec.astype(np.float32),
            resets=np.tile(pp["resets"][c][None, :].astype(bf16), (P, 1)),
            endi=endw,
            bncor=bncor,
            ident=ident,
            identf=identf,
            cntv=pp["cnt"][c][None, :].astype(bf16),
            dinv=pp["dinv_nm"][c],
        ))

    trace = bool(os.environ.get("KERNEL_TRACE"))
    res = run_bass_kernel_spmd(nc, in_maps, list(range(NC)), trace=trace)
    kernel.last_exec_ns = res.exec_time_ns
    kernel.last_result = res

    outp = np.zeros((NG, 9 * D), np.float32)
    for c in range(NC):
        oc = res.results[c]["out"].reshape(G_pad, 9 * D)
        Gc = pp["g1"][c] - pp["g0"][c]
        outp[pp["g0"][c]:pp["g1"][c]] = oc[:Gc]
    outp[pp["empty"]] = -np.inf
    return outp


# revision 20
# speedup vs baseline: 1.0060x; 1.0060x over previous
"""Trainium2 Bass kernel v7 for nn_CLGF_GNNDrug (GCN+GIN drug GNN, 8 cores).

Key changes vs v2 baseline (3.18ms):
  - SWDGE dma_gather calls round-robin over 4 SWDGE queues: the single-queue
    ucode stalls on its own ring drain (7.6us/call); 4 queues pipeline prep
    against DMA drain (~2.0us/call, gathers run at ~230GB/s vs ~96GB/s).
  - Sorted-src chunking against HALF tables (2 x 51200 rows) addressed via
    offset-view in_aps (int16 idx is relative to a per-call base row), giving
    ~96% chunk fill vs 67% for the old (tile, quarter) cell scheme. Edge rows
    per layer drop from ~113k to ~80k.
  - Tables AllGathered in 2 chunks/layer (halves), overlapped with compute.
"""
import os
import sys
import types

import numpy as np
import ml_dtypes


def _install_ntff_hook():
    try:
        from antenv.axon_hooks import get_axon_ntff_profile_hook  # noqa: F401
        return
    except ImportError:
        pass
    try:
        from trn_agent_boot.trn_boot import _ntff_profile_via_ctypes
        hook = _ntff_profile_via_ctypes("/opt/axon/libaxon_pjrt.so")
    except Exception:
        hook = None
    mod = types.ModuleType("antenv.axon_hooks")
    mod.get_axon_ntff_profile_hook = lambda: hook
    mod.set_axon_ntff_profile_hook = lambda h: None
    sys.modules["antenv.axon_hooks"] = mod


_install_ntff_hook()

import concourse.bass as bass
import concourse.bacc as bacc
import concourse.mybir as mybir
import concourse.tile as tile
from concourse.bass_utils import run_bass_kernel_spmd

N = 100000
E = 500000
NG = 4000
F_IN = 77
D = 128
BN_EPS = 1e-5
NC = 8
P = 128
GRP = 512
GPT = GRP // P
NH = 2             # half tables per layer
NEG = -1.0e30
MAXCH = 7          # max chunks (x128 idxs) per dma_gather call
WIN = 32768        # int16 idx window (rows) per gather call
NQUEUE = 4         # SWDGE queues

dt = mybir.dt
BF = dt.float16
F32 = dt.float32
bf16 = np.float16


# ============================= host preprocessing =============================

def prep(x, edge_index, batch):
    x = np.asarray(x, np.float32)
    src_all = np.asarray(edge_index[0], np.int64)
    dst_all = np.asarray(edge_index[1], np.int64)
    batch = np.asarray(batch, np.int64)

    gsizes = np.bincount(batch, minlength=NG)
    gstart = np.concatenate([[0], np.cumsum(gsizes)])
    cuts = [0]
    for c in range(1, NC):
        target = c * N // NC
        g = int(np.searchsorted(gstart, target))
        if g > 0 and abs(gstart[g - 1] - target) < abs(gstart[min(g, NG)] - target):
            g -= 1
        g = min(max(g, cuts[-1]), NG)
        cuts.append(g)
    cuts.append(NG)
    g0 = np.array(cuts[:-1]); g1 = np.array(cuts[1:])
    n0 = gstart[g0]; n1 = gstart[g1]
    ncore = (n1 - n0).astype(np.int64)

    S = int(np.ceil(ncore.max() / GRP) * GRP)
    T = S // P
    NGRP = S // GRP
    HR = S // NH           # local rows per half
    NR = NC * HR           # rows per half-table
    Gc = (g1 - g0).astype(np.int64)
    G_pad = int(np.ceil((Gc.max() + 1) / 16) * 16)

    core_of = np.searchsorted(n1, np.arange(N), side="right")
    local = np.arange(N) - n0[core_of]
    shalf = local // HR                    # source half of each node
    srel = core_of * HR + (local % HR)     # row within half-table

    deg = 1.0 + np.bincount(dst_all, minlength=N).astype(np.float64)
    dinv = (1.0 / np.sqrt(deg)).astype(np.float32)

    # regular edges only — self-loops are handled via dedicated per-tile
    # "self chunks" whose rhs is a contiguous load from this core's own
    # emitted rows (ag buffers), not a gather.
    es = src_all
    ed = dst_all

    ecore = core_of[ed]
    dloc = local[ed]
    tl = dloc // P
    fc = (dloc % P).astype(np.int64)
    hh = shalf[es]
    rr = srel[es]

    # per-core edge lists sorted by (cell, table row)
    pc = {}
    for c in range(NC):
        idxs = np.where(ecore == c)[0]
        k_c = tl[idxs] * NH + hh[idxs]
        o = np.lexsort((rr[idxs], k_c))
        idxs = idxs[o]; k_c = k_c[o]
        cnts = np.bincount(k_c, minlength=T * NH)
        cb = np.concatenate([[0], np.cumsum(cnts)])[:-1]
        pc[c] = (idxs, k_c, cnts, cb)

    # shared (SPMD-uniform) segment cuts per cell: cut whenever any core
    # would exceed 128 rows in the segment, or the row span would exceed
    # the int16 gather window.
    nch = np.zeros((T, NH), np.int64)
    cell_cuts = {}
    for t in range(T):
        for h in range(NH):
            k = t * NH + h
            lists = [rr[pc[c][0][pc[c][3][k]:pc[c][3][k] + pc[c][2][k]]]
                     for c in range(NC)]
            ptr = [0] * NC
            cuts = []
            while True:
                rem = [lst[q:] for lst, q in zip(lists, ptr)]
                if all(len(r) == 0 for r in rem):
                    break
                first = min(int(r[0]) for r in rem if len(r))
                cut = first + WIN          # exclusive upper bound
                for r in rem:
                    if len(r) > P:
                        cut = min(cut, int(r[P]))
                assert cut > first, "degenerate segment (>128 equal rows)"
                cuts.append(cut)
                for ci, lst in enumerate(lists):
                    ptr[ci] += int(np.searchsorted(lst[ptr[ci]:], cut))
            cell_cuts[k] = np.array(cuts if cuts else [1], np.int64)
            nch[t, h] = len(cuts)
    MAXJ = max(int(nch.max()), 1)

    # per-core slot assignment + chunk row ranges
    cmin = np.full((T, NH, MAXJ), np.iinfo(np.int64).max, np.int64)
    cmax = np.full((T, NH, MAXJ), -1, np.int64)
    order_all = {}
    for c in range(NC):
        idxs, k_c, cnts, cb = pc[c]
        j = np.zeros(len(idxs), np.int64)
        for k in np.unique(k_c):
            sl = slice(cb[k], cb[k] + cnts[k])
            j[sl] = np.searchsorted(cell_cuts[k], rr[idxs[sl]], side="right")
        key2 = k_c * MAXJ + j
        gs = np.ones(len(idxs), bool)
        gs[1:] = key2[1:] != key2[:-1]
        startidx = np.maximum.accumulate(np.where(gs, np.arange(len(idxs)), 0))
        rank = np.arange(len(idxs)) - startidx
        assert len(rank) == 0 or rank.max() < P
        order_all[c] = (idxs, k_c, j, rank)
        np.minimum.at(cmin.reshape(-1), key2, rr[idxs])
        np.maximum.at(cmax.reshape(-1), key2, rr[idxs])
    empty = cmax.reshape(-1) < 0
    cmin.reshape(-1)[empty] = 0
    cmax.reshape(-1)[empty] = 0

    # call packing per (group, half): greedy by ascending min row.
    # Self chunks (one per tile, rhs loaded contiguously from ag buffers)
    # come first in each group.
    chunkpos = np.full((T, NH, MAXJ), -1, np.int64)
    basearr = []          # per chunk position: call base row
    band_of = []
    calls_by_g = [[] for _ in range(NGRP)]
    selfs_by_g = [[] for _ in range(NGRP)]
    selfpos = np.zeros(T, np.int64)
    pos = 0
    gs0 = []; gcnt = []
    for g in range(NGRP):
        gs0.append(pos)
        for t in range(g * GPT, (g + 1) * GPT):
            k_half = (t * P) // HR
            r0 = t * P - k_half * HR
            selfpos[t] = pos
            selfs_by_g[g].append((pos, t, k_half, r0))
            basearr.append(0)
            band_of.append(t % GPT)
            pos += 1
        for h in range(NH):
            chunks = []
            for t in range(g * GPT, (g + 1) * GPT):
                for j in range(int(nch[t, h])):
                    chunks.append((int(cmin[t, h, j]), int(cmax[t, h, j]), t, j))
            chunks.sort()
            cur = []
            cur_base = 0
            cur_max = 0

            def flush():
                nonlocal pos, cur
                if not cur:
                    return
                pos0 = pos
                for (mn, mx, t, j) in cur:
                    chunkpos[t, h, j] = pos
                    basearr.append(cur_base)
                    band_of.append(t % GPT)
                    pos += 1
                calls_by_g[g].append((h, pos0, len(cur), cur_base))
                cur = []

            for (mn, mx, t, j) in chunks:
                if cur and (len(cur) >= MAXCH or mx - cur_base > WIN - 1):
                    flush()
                if not cur:
                    cur_base = mn
                    cur_max = mx
                cur_max = max(cur_max, mx)
                cur.append((mn, mx, t, j))
                assert cur_max - cur_base <= WIN - 1
            flush()
        gcnt.append(pos - gs0[-1])
    C = pos
    basearr = np.array(basearr, np.int64)
    band_of = np.array(band_of, np.int64)

    # per-tile matmul order: self chunk first, then gather chunks
    tile_chunks = []
    for t in range(T):
        lst = [int(selfpos[t])] + sorted(
            int(p) for p in chunkpos[t].reshape(-1) if p >= 0)
        tile_chunks.append(lst)
    # per-group position count of selfs + half-0 calls (available before the
    # half-1 AllGather)
    n_h0 = []
    for g in range(NGRP):
        n = GPT + sum(nck for (h, p0, nck, b) in calls_by_g[g] if h == 0)
        n_h0.append(n)

    # per-core slot data
    xrow = np.zeros((N, 2 * P), bf16)
    xrow[:, :F_IN] = (dinv[:, None] * x).astype(bf16)
    xrow[:, P:P + F_IN] = x.astype(bf16)

    src16 = np.zeros((NC, C * P), np.int16)
    m = np.zeros((NC, P, C * P), bf16)
    xe = np.zeros((NC, P, C * 2 * P), bf16)
    for c in range(NC):
        idxs, k_c, j, rank = order_all[c]
        p = rank
        t_c = k_c // NH
        h_c = k_c % NH
        pos_e = chunkpos[t_c, h_c, j]
        assert (pos_e >= 0).all()
        rel = rr[idxs] - basearr[pos_e]
        if len(rel):
            assert rel.min() >= 0 and rel.max() <= WIN - 1
        src16[c, pos_e * P + p] = rel.astype(np.int16)
        m[c, p, pos_e * P + fc[idxs]] = 1.0
        xe[c].reshape(P, C, 2 * P)[p, pos_e, :] = xrow[es[idxs]]
        # self-loop chunks: node (t*P + p) at slot p of selfpos[t]
        nreal = int(ncore[c])
        loc = np.arange(nreal)
        tt = loc // P
        ps = loc % P
        spos = selfpos[tt]
        m[c, ps, spos * P + ps] = 1.0
        xe[c].reshape(P, C, 2 * P)[ps, spos, :] = xrow[n0[c] + loc]

    # wrapped int16 idx packing (idx n of chunk k -> [n%16, k*8 + n//16])
    idx16 = np.zeros((NC, P, C * 8), np.int16)
    for c in range(NC):
        w = src16[c].reshape(C * 8, 16).T
        idx16[c] = np.tile(w, (8, 1))

    resets = np.zeros((NC, S), np.float32)
    end_ids = np.zeros((NC, G_pad), np.int64)
    end_par = np.zeros((NC, G_pad), np.float32)
    for c in range(NC):
        gs = gstart[g0[c]:g1[c] + 1] - n0[c]
        starts = gs[:-1]; ends = gs[1:] - 1
        ne = gsizes[g0[c]:g1[c]] > 0
        resets[c, starts[ne]] = NEG
        if ncore[c] < S:
            resets[c, ncore[c]] = NEG
        end_ids[c, :g1[c] - g0[c]][ne] = ends[ne] // 2
        end_par[c, :g1[c] - g0[c]][ne] = (ends[ne] % 2).astype(np.float32)

    cntdeg = np.zeros((NC, S), np.float32)
    dinv_nm = np.ones((NC, P, T), np.float32)
    for c in range(NC):
        cntdeg[c, :ncore[c]] = deg[n0[c]:n1[c]].astype(np.float32)
        dv = dinv[n0[c]:n1[c]]
        dpad = np.ones(S, np.float32)
        dpad[:ncore[c]] = dv
        dinv_nm[c] = dpad.reshape(T, P).T

    return dict(
        S=S, T=T, NGRP=NGRP, HR=HR, NR=NR, C=C, G_pad=G_pad,
        calls_by_g=calls_by_g, selfs_by_g=selfs_by_g,
        gs0=gs0, gcnt=gcnt, band_of=band_of,
        tile_chunks=tile_chunks, n_h0=n_h0,
        g0=g0, g1=g1, n0=n0, n1=n1, ncore=ncore,
        src16=src16, m=m, xe=xe, idx16=idx16,
        resets=resets, end_ids=end_ids, end_par=end_par,
        empty=(gsizes == 0), cnt=cntdeg, dinv_nm=dinv_nm,
    )


# ============================= device program =============================

def build_program(meta, debug=False):
    S = meta["S"]; T = meta["T"]; NGRP = meta["NGRP"]
    HR = meta["HR"]; NR = meta["NR"]; C = meta["C"]; G_pad = meta["G_pad"]
    calls_by_g = meta["calls_by_g"]
    selfs_by_g = meta["selfs_by_g"]
    gs0 = meta["gs0"]; gcnt = meta["gcnt"]
    tile_chunks = meta["tile_chunks"]; n_h0 = meta["n_h0"]

    nc = bacc.Bacc("TRN2", target_bir_lowering=False, num_swdge_queues=NQUEUE)
    AluOp = mybir.AluOpType
    Act = mybir.ActivationFunctionType

    xe_d = nc.dram_tensor("xe", [P, C * 2 * P], BF, kind="ExternalInput")
    m_d = nc.dram_tensor("m", [P, C * P], BF, kind="ExternalInput")
    idx_d = nc.dram_tensor("idx16", [P, C * 8], dt.int16, kind="ExternalInput")
    wstk = nc.dram_tensor("wstk", [8 * P, D], BF, kind="ExternalInput")
    pvec = nc.dram_tensor("pvec", [P, 14], F32, kind="ExternalInput")
    rst_d = nc.dram_tensor("resets", [P, S], BF, kind="ExternalInput")
    endi = nc.dram_tensor("endi", [P, G_pad // 16], dt.int16, kind="ExternalInput")
    bncor = nc.dram_tensor("bncor", [P, 6], F32, kind="ExternalInput")
    ident = nc.dram_tensor("ident", [P, P], BF, kind="ExternalInput")
    identf = nc.dram_tensor("identf", [P, P], F32, kind="ExternalInput")
    cntv = nc.dram_tensor("cntv", [1, S], BF, kind="ExternalInput")
    dinv_d = nc.dram_tensor("dinv", [P, T], F32, kind="ExternalInput")
    parw_d = nc.dram_tensor("parw", [P, G_pad], F32, kind="ExternalInput")

    out = nc.dram_tensor("out", [G_pad * 9, D], F32, kind="ExternalOutput")

    ag2h = [nc.dram_tensor(f"ag2_{k}", [HR, 2 * D], BF) for k in range(NH)]
    tab2h = [nc.dram_tensor(f"tab2_{k}", [NR, 2 * D], BF, addr_space="Shared")
             for k in range(NH)]
    ag3h = [nc.dram_tensor(f"ag3_{k}", [HR, D], BF) for k in range(NH)]
    tab3h = [nc.dram_tensor(f"tab3_{k}", [NR, D], BF, addr_space="Shared")
             for k in range(NH)]
    bn_in = [nc.dram_tensor(f"bn{i}_in", [P, 2], F32) for i in range(3)]
    bn_out = [nc.dram_tensor(f"bn{i}_out", [P, 2], F32, addr_space="Shared")
              for i in range(3)]
    SL = {}
    slkind = dict(kind="ExternalOutput") if debug else {}
    for nme in ("xg1", "xg2", "u0", "u1", "u2"):
        SL[nme] = nc.dram_tensor(f"sl_{nme}", [P, S], BF, **slkind)
    if debug:
        dbg_scan = nc.dram_tensor("dbg_scan", [P, 2 * S], BF,
                                  kind="ExternalOutput")
        dbg_ext = nc.dram_tensor("dbg_ext", [P, 4 * G_pad], BF,
                                 kind="ExternalOutput")
        dbg_pool = nc.dram_tensor("dbg_pool", [P, G_pad * 9], F32,
                                  kind="ExternalOutput")

    RG = [list(range(NC))]
    # AllGather chunk trigger group: half k ready after group trig[k]
    trig = {}
    for k in range(NH):
        trig[((k + 1) * HR - 1) // GRP] = k

    qctr = [0]

    def next_q():
        q = qctr[0] % NQUEUE
        qctr[0] += 1
        return q

    with tile.TileContext(nc) as tc:
        with (
            tc.tile_pool(name="cst", bufs=1) as cst,
            tc.tile_pool(name="scn", bufs=3) as scnp,
            tc.tile_pool(name="gat", bufs=2) as gat,
            tc.tile_pool(name="mbuf", bufs=2) as mbp,
            tc.tile_pool(name="work", bufs=2) as wkp,
            tc.tile_pool(name="one", bufs=1) as onep,
            tc.tile_pool(name="ps2b", bufs=4, space="PSUM") as psa_p,
            tc.tile_pool(name="psd", bufs=1, space="PSUM") as psd,
            tc.tile_pool(name="pst", bufs=2, space="PSUM") as pst,
        ):
            # ---------------- constants ----------------
            w_sb = cst.tile([P, 8 * D], BF)
            for i in range(8):
                nc.sync.dma_start(out=w_sb[:, i * D:(i + 1) * D],
                                  in_=wstk[i * P:(i + 1) * P, :])
            pv = cst.tile([P, 14], F32)
            nc.sync.dma_start(out=pv[:], in_=pvec[:, :])
            endi_sb = cst.tile([P, G_pad // 16], dt.int16)
            nc.sync.dma_start(out=endi_sb[:], in_=endi[:, :])
            idn = cst.tile([P, P], BF)
            nc.sync.dma_start(out=idn[:], in_=ident[:, :])
            idnf = cst.tile([P, P], F32)
            nc.sync.dma_start(out=idnf[:], in_=identf[:, :])
            bnc = cst.tile([P, 6], F32)
            nc.sync.dma_start(out=bnc[:], in_=bncor[:, :])
            dinv_sb = cst.tile([P, T], F32)
            nc.sync.dma_start(out=dinv_sb[:], in_=dinv_d[:, :])
            parw_sb = cst.tile([P, G_pad], F32)
            nc.sync.dma_start(out=parw_sb[:], in_=parw_d[:, :])
            aeff_sb = cst.tile([P, 2 * D], BF)
            atv_row = cst.tile([1, 2 * P], BF)

            W1, W2 = w_sb[:, 0:D], w_sb[:, D:2 * D]
            A = [w_sb[:, (2 + 2 * i) * D:(3 + 2 * i) * D] for i in range(3)]
            B = [w_sb[:, (3 + 2 * i) * D:(4 + 2 * i) * D] for i in range(3)]
            b1c, b2c = pv[:, 0:1], pv[:, 1:2]
            ac = [pv[:, 2 + 4 * i:3 + 4 * i] for i in range(3)]
            cc = [pv[:, 3 + 4 * i:4 + 4 * i] for i in range(3)]
            gcl = [pv[:, 4 + 4 * i:5 + 4 * i] for i in range(3)]
            bec = [pv[:, 5 + 4 * i:6 + 4 * i] for i in range(3)]

            pooled = cst.tile([P, G_pad * 9], F32)
            stats = cst.tile([P, 3 * 2 * NGRP], F32)
            sf = cst.tile([P, 8], F32)
            bnreg = cst.tile([P, 9], F32)

            scanbufs = {}
            prevs = {}

            def load_rst(g):
                rt = wkp.tile([P, GRP], BF, tag="rstg", name=f"rst_{g}")
                nc.sync.dma_start(out=rt[:],
                                  in_=rst_d[:, g * GRP:(g + 1) * GRP])
                return rt

            def scan_g(comp, g, data1, rt):
                if comp not in scanbufs:
                    scanbufs[comp] = scnp.tile([P, S], BF, tag="scan",
                                               name=f"scan_c{comp}")
                    prevs[comp] = 0.0
                sb = scanbufs[comp]
                nc.vector.tensor_tensor_scan(
                    out=sb[:, g * GRP:(g + 1) * GRP],
                    data0=rt[:],
                    data1=data1,
                    initial=prevs[comp],
                    op0=AluOp.add,
                    op1=AluOp.max)
                prevs[comp] = sb[:, (g + 1) * GRP - 1:(g + 1) * GRP]

            def extract(comp):
                # bf16 ap_gather needs d=2: gather the PAIR containing the
                # graph-end column, then select the half via the parity mask.
                sb = scanbufs[comp]
                ext = wkp.tile([P, G_pad * 2], BF, tag="nm",
                                name=f"ext{comp}")
                nc.gpsimd.ap_gather(
                    out_ap=ext[:].rearrange("p (g o) -> p g o", o=2),
                    in_ap=sb[:].rearrange("p (s o) -> p s o", o=2),
                    idxs_ap=endi_sb[:], channels=P, num_elems=S // 2, d=2,
                    num_idxs=G_pad)
                pv_ = pooled[:, comp::9]
                nc.vector.tensor_tensor(out=pv_, in0=ext[:, 1::2],
                                        in1=ext[:, 0::2], op=AluOp.subtract)
                nc.vector.tensor_tensor(out=pv_, in0=pv_, in1=parw_sb[:],
                                        op=AluOp.mult)
                nc.vector.tensor_tensor(out=pv_, in0=pv_, in1=ext[:, 0::2],
                                        op=AluOp.add)
                if debug and comp in (0, 4):
                    hf = comp // 4
                    nc.sync.dma_start(out=dbg_scan[:, hf * S:(hf + 1) * S],
                                      in_=sb[:])
                    nc.sync.dma_start(
                        out=dbg_ext[:, hf * 2 * G_pad:(hf + 1) * 2 * G_pad],
                        in_=ext[:])

            # PSUM bands live at 2KB (bank) strides: a matmul with start=True
            # zeroes its entire PSUM bank, so two open accumulations must
            # never share a bank.
            BANDW = 512  # f32 elems per band slot = one 2KB bank

            def agg_tiles(g, rhs0, rhs1, split, elem, halves):
                """Per-tile band accumulation -> feature-major agg tiles.

                Phase A issues every tile's half-0 (selfs + early-table)
                matmuls first, so the PE keeps working while the half-1
                AllGather is still in flight; phase A2 adds the half-1
                matmuls; phase B drains the bands (copies + transposes).
                Bands rotate through 4 PSUM banks."""
                s0 = gs0[g]
                aggs = {}
                for nme, _ in halves:
                    aggs[nme] = wkp.tile([P, GRP], BF, tag=f"agg{nme}",
                                         name=f"agg_{nme}_{g}")
                bands = []
                for ti in range(GPT):
                    t = g * GPT + ti
                    lst = tile_chunks[t]
                    lst0 = [p for p in lst if p - s0 < split]
                    lst1 = [p for p in lst if p - s0 >= split]
                    band = psa_p.tile([P, BANDW], F32, tag="psa",
                                      name=f"psa_{elem}_{g}_{ti}")
                    bands.append((band, lst0, lst1))
                    for i, pos in enumerate(lst0):
                        off = pos - s0
                        nc.tensor.matmul(
                            band[:, 0:elem],
                            m_t[:, off * P:(off + 1) * P],
                            rhs0[:, off * elem:(off + 1) * elem],
                            start=(i == 0),
                            stop=(not lst1 and i == len(lst0) - 1))
                for ti in range(GPT):
                    band, lst0, lst1 = bands[ti]
                    for i, pos in enumerate(lst1):
                        off = pos - s0
                        o2 = off - split
                        nc.tensor.matmul(
                            band[:, 0:elem],
                            m_t[:, off * P:(off + 1) * P],
                            rhs1[:, o2 * elem:(o2 + 1) * elem],
                            start=False, stop=(i == len(lst1) - 1))
                for ti in range(GPT):
                    t = g * GPT + ti
                    band = bands[ti][0]
                    col = dinv_sb[:, t:t + 1]
                    nm = wkp.tile([P, len(halves) * P], BF, tag="nm",
                                  name=f"nm_{elem}_{g}_{ti}")
                    for hi, (nme, scaled) in enumerate(halves):
                        src = band[:, hi * P:(hi + 1) * P]
                        dst = nm[:, hi * P:(hi + 1) * P]
                        if scaled:
                            nc.scalar.activation(dst, src, Act.Identity,
                                                 scale=col)
                        elif hi % 2 == 0:
                            nc.scalar.copy(dst, src)
                        else:
                            nc.vector.tensor_copy(dst, src)
                    for hi, (nme, _) in enumerate(halves):
                        ptile = pst.tile([P, P], BF, tag="tp",
                                         name=f"tp_{nme}_{g}_{ti}")
                        nc.tensor.transpose(
                            ptile[:], nm[:, hi * P:(hi + 1) * P], idn[:])
                        if ti % 2 == 0:
                            nc.vector.tensor_copy(
                                aggs[nme][:, ti * P:(ti + 1) * P], ptile[:])
                        else:
                            nc.scalar.copy(
                                aggs[nme][:, ti * P:(ti + 1) * P], ptile[:])
                return aggs

            def emit_nm(g, o_tile, dest_list, col0, width, scale_dinv):
                """Transpose feature-major dense output to node-major rows and
                DMA into the collective-input tensors (split at HR bounds)."""
                em = wkp.tile([P, GPT * P], BF, tag="em", name=f"em_{g}_{col0}")
                for t in range(GPT):
                    ptile = pst.tile([P, P], BF, tag="tp",
                                     name=f"em_tp_{g}_{col0}_{t}")
                    nc.tensor.transpose(ptile[:], o_tile[:, t * P:(t + 1) * P],
                                        idn[:])
                    if scale_dinv:
                        nc.scalar.activation(
                            em[:, t * P:(t + 1) * P], ptile[:], Act.Identity,
                            scale=dinv_sb[:, g * GPT + t:g * GPT + t + 1])
                    else:
                        nc.scalar.copy(em[:, t * P:(t + 1) * P], ptile[:])
                # DMA node-major: rows g*GRP + t*128 + p
                r0 = g * GRP
                t0 = 0
                while t0 < GPT:
                    k = (r0 + t0 * P) // HR
                    tmax = min(GPT, ((k + 1) * HR - r0) // P)
                    nt = tmax - t0
                    dest = dest_list[k]
                    rr_ = r0 + t0 * P - k * HR
                    dst_ap = dest[rr_:rr_ + nt * P, col0:col0 + width]
                    dst_ap = dst_ap.rearrange("(t p) f -> p t f", p=P)
                    src_ap = em[:, t0 * P:(t0 + nt) * P]
                    src_ap = src_ap.rearrange("p (t f) -> p t f", f=P)
                    nc.sync.dma_start(out=dst_ap, in_=src_ap)
                    t0 = tmax

            def dense_gcn(g, aggbuf, W, bcol, sl_dest):
                ps = psd.tile([P, GRP], F32, tag="zd")
                nc.tensor.matmul(ps[:], W, aggbuf[:], start=True, stop=True)
                o = wkp.tile([P, GRP], BF, tag="obf", name=f"ogcn_{g}")
                nc.scalar.activation(o[:], ps[:], Act.Relu, bias=bcol)
                nc.sync.dma_start(
                    out=sl_dest[:, g * GRP:(g + 1) * GRP], in_=o[:])
                return o

            def dense_gin(g, aggbuf, li, sl_dest):
                ps1 = psd.tile([P, GRP], F32, tag="zd")
                if li == 0:
                    nc.tensor.matmul(ps1[:], A[0], aggbuf[:],
                                     start=True, stop=True)
                else:
                    Aeff = aeff_sb[:, (li - 1) * D:li * D]
                    nc.tensor.matmul(ps1[:], Aeff, aggbuf[:],
                                     start=True, stop=False)
                    cg = wkp.tile([1, GRP], BF, tag="cntg",
                                  name=f"cg{li}_{g}")
                    nc.sync.dma_start(
                        out=cg[:], in_=cntv[0:1, g * GRP:(g + 1) * GRP])
                    nc.tensor.matmul(
                        ps1[:], atv_row[0:1, (li - 1) * P:li * P],
                        cg[0:1, :], start=False, stop=True)
                ua = wkp.tile([P, GRP], BF, tag="ua", name=f"ua{li}_{g}")
                nc.scalar.activation(ua[:], ps1[:], Act.Relu, bias=ac[li])
                ps2 = psd.tile([P, GRP], F32, tag="zd2")
                nc.tensor.matmul(ps2[:], B[li], ua[:], start=True, stop=True)
                o32 = wkp.tile([P, GRP], F32, tag="o32", name=f"og{li}_{g}")
                base = li * 2 * NGRP
                nc.scalar.activation(o32[:], ps2[:], Act.Relu, bias=cc[li],
                                     accum_out=stats[:, base + 2 * g:
                                                     base + 2 * g + 1])
                sq = onep.tile([P, GRP], F32, tag="sq", name=f"sq{li}_{g}")
                nc.vector.tensor_tensor(out=sq[:], in0=o32[:], in1=o32[:],
                                        op=AluOp.mult)
                nc.vector.reduce_sum(
                    stats[:, base + 2 * g + 1:base + 2 * g + 2], sq[:],
                    axis=mybir.AxisListType.X)
                obf = wkp.tile([P, GRP], BF, tag="obf", name=f"ogb{li}_{g}")
                nc.vector.tensor_copy(obf[:], o32[:])
                nc.sync.dma_start(
                    out=sl_dest[:, g * GRP:(g + 1) * GRP], in_=obf[:])
                return o32, obf

            def bn_finalize(li):
                base = li * 2 * NGRP
                nc.vector.reduce_sum(sf[:, 0:1], stats[:, base:base + 2 * NGRP:2],
                                     axis=mybir.AxisListType.X)
                nc.vector.reduce_sum(sf[:, 1:2],
                                     stats[:, base + 1:base + 2 * NGRP:2],
                                     axis=mybir.AxisListType.X)
                nc.vector.tensor_tensor(out=sf[:, 0:2], in0=sf[:, 0:2],
                                        in1=bnc[:, 2 * li:2 * li + 2],
                                        op=AluOp.subtract)
                nc.sync.dma_start(out=bn_in[li][:, :], in_=sf[:, 0:2])
                nc.gpsimd.collective_compute(
                    "AllReduce", AluOp.add, replica_groups=RG,
                    ins=[bn_in[li][:, :]], outs=[bn_out[li][:, :]])
                nc.sync.dma_start(out=sf[:, 2:4], in_=bn_out[li][:, :])
                nc.vector.tensor_scalar(out=sf[:, 4:5], in0=sf[:, 2:3],
                                        scalar1=1.0 / N, scalar2=None,
                                        op0=AluOp.mult)
                nc.vector.tensor_scalar(out=sf[:, 5:6], in0=sf[:, 3:4],
                                        scalar1=1.0 / N, scalar2=None,
                                        op0=AluOp.mult)
                nc.vector.tensor_tensor(out=sf[:, 6:7], in0=sf[:, 4:5],
                                        in1=sf[:, 4:5], op=AluOp.mult)
                nc.vector.tensor_tensor(out=sf[:, 5:6], in0=sf[:, 5:6],
                                        in1=sf[:, 6:7], op=AluOp.subtract)
                nc.vector.tensor_scalar(out=sf[:, 5:6], in0=sf[:, 5:6],
                                        scalar1=BN_EPS, scalar2=None,
                                        op0=AluOp.add)
                nc.scalar.activation(sf[:, 5:6], sf[:, 5:6], Act.Sqrt)
                nc.vector.reciprocal(sf[:, 6:7], sf[:, 5:6])
                nc.vector.tensor_tensor(out=sf[:, 6:7], in0=sf[:, 6:7],
                                        in1=gcl[li], op=AluOp.mult)
                nc.vector.tensor_tensor(out=sf[:, 7:8], in0=sf[:, 4:5],
                                        in1=sf[:, 6:7], op=AluOp.mult)
                nc.vector.tensor_tensor(out=sf[:, 7:8], in0=bec[li],
                                        in1=sf[:, 7:8], op=AluOp.subtract)
                scol = bnreg[:, 3 * li:3 * li + 1]
                tcol = bnreg[:, 3 * li + 1:3 * li + 2]
                rcol = bnreg[:, 3 * li + 2:3 * li + 3]
                nc.vector.tensor_copy(scol, sf[:, 6:7])
                nc.vector.tensor_copy(tcol, sf[:, 7:8])
                nc.vector.reciprocal(sf[:, 0:1], sf[:, 6:7])
                nc.vector.tensor_tensor(out=rcol, in0=sf[:, 7:8],
                                        in1=sf[:, 0:1], op=AluOp.mult)
                if li < 2:
                    nc.vector.tensor_scalar(
                        out=aeff_sb[:, li * D:(li + 1) * D], in0=A[li + 1],
                        scalar1=sf[:, 6:7], scalar2=None, op0=AluOp.mult)
                    tbf = wkp.tile([P, 1], BF, tag="nm1", name=f"tbf{li}")
                    nc.vector.tensor_copy(tbf[:], sf[:, 7:8])
                    pv1 = pst.tile([P, 1], F32, tag="tp", name=f"atv{li}")
                    nc.tensor.matmul(pv1[:], A[li + 1], tbf[:],
                                     start=True, stop=True)
                    atc = wkp.tile([P, 1], BF, tag="nm1", name=f"atc{li}")
                    nc.vector.tensor_copy(atc[:], pv1[:])
                    pv2 = pst.tile([1, P], BF, tag="tp", name=f"atr{li}")
                    nc.tensor.transpose(pv2[:], atc[:], idn[:])
                    nc.vector.tensor_copy(atv_row[0:1, li * P:(li + 1) * P],
                                          pv2[:])

            def issue_h0(g, ag_list, tabs, elem):
                split = n_h0[g]
                gt0 = gat.tile([P, split * elem], BF, tag="g0",
                               name=f"g0{elem}_{g}")
                s0 = gs0[g]
                # self chunks: contiguous load of this core's own emitted rows
                for (pos, t, kh, r0) in selfs_by_g[g]:
                    off = pos - s0
                    nc.sync.dma_start(
                        out=gt0[:, off * elem:(off + 1) * elem],
                        in_=ag_list[kh][r0:r0 + P, :])
                idx_t = wkp.tile([P, gcnt[g] * 8], dt.int16, tag="idxg2",
                                 name=f"idx_{elem}_{g}")
                nc.sync.dma_start(
                    out=idx_t[:], in_=idx_d[:, s0 * 8:(s0 + gcnt[g]) * 8])
                for (h, pos0, nck, cbase) in calls_by_g[g]:
                    if h != 0:
                        continue
                    o0 = pos0 - s0
                    n = nck * P
                    W = min(WIN, NR - cbase)
                    nview = gt0[:, o0 * elem:(o0 + nck) * elem]
                    nc.gpsimd.dma_gather(
                        nview.rearrange("p (j e) -> p j e", e=elem),
                        tabs[0][cbase:cbase + W, :],
                        idx_t[:, o0 * 8:(o0 + nck) * 8],
                        n, n, elem, queue_num=next_q(),
                    )
                return gt0, idx_t

            def issue_h1(g, tabs, elem, idx_t):
                split = n_h0[g]
                gt1 = gat.tile([P, (gcnt[g] - split) * elem], BF, tag="g1",
                               name=f"g1{elem}_{g}")
                s0 = gs0[g]
                for (h, pos0, nck, cbase) in calls_by_g[g]:
                    if h != 1:
                        continue
                    o0 = pos0 - s0
                    o1 = o0 - split
                    n = nck * P
                    W = min(WIN, NR - cbase)
                    nview = gt1[:, o1 * elem:(o1 + nck) * elem]
                    nc.gpsimd.dma_gather(
                        nview.rearrange("p (j e) -> p j e", e=elem),
                        tabs[1][cbase:cbase + W, :],
                        idx_t[:, o0 * 8:(o0 + nck) * 8],
                        n, n, elem, queue_num=next_q(),
                    )
                return gt1

            def load_m(g):
                mt = mbp.tile([P, gcnt[g] * P], BF, tag="m", name=f"m_{g}")
                nc.sync.dma_start(
                    out=mt[:], in_=m_d[:, gs0[g] * P:(gs0[g] + gcnt[g]) * P])
                return mt

            def load_sl(nme, g, tag):
                st = wkp.tile([P, GRP], BF, tag=tag, name=f"ld_{nme}_{g}")
                nc.sync.dma_start(out=st[:],
                                  in_=SL[nme][:, g * GRP:(g + 1) * GRP])
                return st

            # ================= LAYER 1 =================
            for g in range(NGRP):
                xe_t = gat.tile([P, gcnt[g] * 2 * P], BF, tag="g0",
                                name=f"xe_{g}")
                nc.sync.dma_start(
                    out=xe_t[:],
                    in_=xe_d[:, gs0[g] * 2 * P:(gs0[g] + gcnt[g]) * 2 * P])
                m_t = load_m(g)
                aggs = agg_tiles(g, xe_t, xe_t, gcnt[g], 2 * P,
                                 [("gc", True), ("gi", False)])
                rt = load_rst(g)
                o_xg1 = dense_gcn(g, aggs["gc"], W1, b1c, SL["xg1"])
                scan_g(0, g, o_xg1[:], rt)
                emit_nm(g, o_xg1, ag2h, 0, D, True)
                o_u0, o_u0b = dense_gin(g, aggs["gi"], 0, SL["u0"])
                scan_g(4, g, o_u0[:], rt)
                emit_nm(g, o_u0b, ag2h, D, D, False)
                if g in trig and trig[g] < NH - 1:
                    k = trig[g]
                    nc.gpsimd.collective_compute(
                        "AllGather", AluOp.bypass, replica_groups=RG,
                        ins=[ag2h[k][:, :]], outs=[tab2h[k][:, :]])

            nc.gpsimd.collective_compute(
                "AllGather", AluOp.bypass, replica_groups=RG,
                ins=[ag2h[NH - 1][:, :]], outs=[tab2h[NH - 1][:, :]])
            bn_finalize(0)
            extract(0)
            extract(4)

            # ================= LAYER 2 =================
            pend2 = {0: issue_h0(0, ag2h, tab2h, 2 * P)}
            for g in range(NGRP):
                if g + 1 < NGRP:
                    pend2[g + 1] = issue_h0(g + 1, ag2h, tab2h, 2 * P)
                gt0, idx_t = pend2.pop(g)
                gt1 = issue_h1(g, tab2h, 2 * P, idx_t)
                m_t = load_m(g)
                aggs = agg_tiles(g, gt0, gt1, n_h0[g], 2 * P,
                                 [("gc", True), ("gi", False)])
                rt = load_rst(g)
                o_xg2 = dense_gcn(g, aggs["gc"], W2, b2c, SL["xg2"])
                scan_g(1, g, o_xg2[:], rt)
                o_u1, o_u1b = dense_gin(g, aggs["gi"], 1, SL["u1"])
                scan_g(5, g, o_u1[:], rt)
                emit_nm(g, o_u1b, ag3h, 0, D, False)
                if g in trig and trig[g] < NH - 1:
                    k = trig[g]
                    nc.gpsimd.collective_compute(
                        "AllGather", AluOp.bypass, replica_groups=RG,
                        ins=[ag3h[k][:, :]], outs=[tab3h[k][:, :]])

            nc.gpsimd.collective_compute(
                "AllGather", AluOp.bypass, replica_groups=RG,
                ins=[ag3h[NH - 1][:, :]], outs=[tab3h[NH - 1][:, :]])
            bn_finalize(1)
            extract(1)
            extract(5)

            s1c = bnreg[:, 0:1]; t1c = bnreg[:, 1:2]; r1c = bnreg[:, 2:3]
            s2c = bnreg[:, 3:4]; t2c = bnreg[:, 4:5]; r2c = bnreg[:, 5:6]
            s3c = bnreg[:, 6:7]; t3c = bnreg[:, 7:8]; r3c = bnreg[:, 8:9]

            # comps 2/3 scans: only need xg1/xg2 — run while tab3h gathers
            # wait on the half-1 AllGather
            for g in range(NGRP):
                rt = load_rst(g)
                xg1t = load_sl("xg1", g, "ldA")
                xg2t = load_sl("xg2", g, "ldB")
                tsum = wkp.tile([P, GRP], BF, tag="tt1", name=f"c2in_{g}")
                nc.vector.tensor_tensor(out=tsum[:], in0=xg1t[:], in1=xg2t[:],
                                        op=AluOp.add)
                scan_g(2, g, tsum[:], rt)
                tprd = wkp.tile([P, GRP], BF, tag="tt2", name=f"c3in_{g}")
                nc.vector.tensor_tensor(out=tprd[:], in0=xg1t[:], in1=xg2t[:],
                                        op=AluOp.mult)
                scan_g(3, g, tprd[:], rt)

            # ================= LAYER 3 =================
            pend3 = {0: issue_h0(0, ag3h, tab3h, P)}
            for g in range(NGRP):
                if g + 1 < NGRP:
                    pend3[g + 1] = issue_h0(g + 1, ag3h, tab3h, P)
                gt0, idx_t = pend3.pop(g)
                gt1 = issue_h1(g, tab3h, P, idx_t)
                m_t = load_m(g)
                aggs = agg_tiles(g, gt0, gt1, n_h0[g], P, [("gi", False)])
                rt = load_rst(g)
                o_u2, _ = dense_gin(g, aggs["gi"], 2, SL["u2"])
                scan_g(6, g, o_u2[:], rt)

            bn_finalize(2)
            extract(2)
            extract(3)
            extract(6)

            # ================= TAIL: comps 7, 8 =================
            for g in range(NGRP):
                u0t = wkp.tile([P, GRP], BF, tag="ldA", name=f"lu0t_{g}")
                nc.sync.dma_start(out=u0t[:],
                                  in_=SL["u0"][:, g * GRP:(g + 1) * GRP])
                u1t = wkp.tile([P, GRP], BF, tag="ldB", name=f"lu1t_{g}")
                nc.sync.dma_start(out=u1t[:],
                                  in_=SL["u1"][:, g * GRP:(g + 1) * GRP])
                u2t = wkp.tile([P, GRP], BF, tag="ldC", name=f"lu2_{g}")
                nc.sync.dma_start(out=u2t[:],
                                  in_=SL["u2"][:, g * GRP:(g + 1) * GRP])
                rt = load_rst(g)
                h1 = wkp.tile([P, GRP], BF, tag="tt3", name=f"h1_{g}")
                nc.scalar.activation(h1[:], u0t[:], Act.Identity, bias=t1c,
                                     scale=s1c)
                h2 = wkp.tile([P, GRP], BF, tag="tt4", name=f"h2_{g}")
                nc.scalar.activation(h2[:], u1t[:], Act.Identity, bias=t2c,
                                     scale=s2c)
                h3 = wkp.tile([P, GRP], BF, tag="tt5", name=f"h3_{g}")
                nc.scalar.activation(h3[:], u2t[:], Act.Identity, bias=t3c,
                                     scale=s3c)
                z3 = wkp.tile([P, GRP], BF, tag="tt6", name=f"z3_{g}")
                nc.vector.tensor_tensor(out=z3[:], in0=h1[:], in1=h2[:],
                                        op=AluOp.add)
                nc.vector.tensor_tensor(out=z3[:], in0=z3[:], in1=h3[:],
                                        op=AluOp.add)
                scan_g(7, g, z3[:], rt)
                w3 = wkp.tile([P, GRP], BF, tag="tt7", name=f"w3_{g}")
                nc.gpsimd.tensor_tensor(out=w3[:], in0=h1[:], in1=h2[:],
                                        op=AluOp.mult)
                nc.gpsimd.tensor_tensor(out=w3[:], in0=w3[:], in1=h3[:],
                                        op=AluOp.mult)
                scan_g(8, g, w3[:], rt)

            extract(7)
            extract(8)

            # pooled-domain BN affine fixups
            for comp, sc, tc_ in ((4, s1c, t1c), (5, s2c, t2c), (6, s3c, t3c)):
                nc.scalar.activation(pooled[:, comp::9], pooled[:, comp::9],
                                     Act.Identity, bias=tc_, scale=sc)

            if debug:
                nc.sync.dma_start(out=dbg_pool[:, :], in_=pooled[:])

            # final transpose-out
            NPT = (G_pad * 9 + P - 1) // P
            for t in range(NPT):
                c0 = t * P
                w = min(P, G_pad * 9 - c0)
                ptile = pst.tile([P, P], F32, tag="tp", name=f"po_{t}")
                nc.tensor.transpose(ptile[:w, :], pooled[:, c0:c0 + w], idnf[:])
                nmo = wkp.tile([P, P], F32, tag="obf", name=f"pon_{t}")
                nc.vector.tensor_copy(nmo[:w, :], ptile[:w, :])
                nc.sync.dma_start(out=out[c0:c0 + w, :], in_=nmo[:w, :])

    nc.finalize()
    return nc


# ============================= top-level kernel =============================

_CACHE = {}


def kernel(x, edge_index, batch, W1, b1, W2, b2,
           A0, a0, B0, c0, g0, be0,
           A1, a1, B1, c1, g1, be1,
           A2, a2, B2, c2, g2, be2):
    pp = prep(x, edge_index, batch)

    debug = bool(os.environ.get("KERNEL_DEBUG"))
    key = (pp["S"], pp["C"], pp["G_pad"], debug,
           repr(pp["calls_by_g"]), pp["band_of"].tobytes())
    if key not in _CACHE:
        _CACHE[key] = build_program(pp, debug=debug)
    nc = _CACHE[key]

    def pad_w(W):
        Wp = np.zeros((P, D), np.float32)
        W = np.asarray(W, np.float32)
        Wp[:W.shape[0]] = W
        return Wp

    wstk = np.concatenate([pad_w(W1), pad_w(W2), pad_w(A0), pad_w(B0),
                           pad_w(A1), pad_w(B1), pad_w(A2), pad_w(B2)],
                          axis=0).astype(bf16)
    pvec = np.stack([np.asarray(v, np.float32) for v in
                     (b1, b2, a0, c0, g0, be0, a1, c1, g1, be1,
                      a2, c2, g2, be2)], axis=1)
    ident = np.eye(P, dtype=bf16)
    identf = np.eye(P, dtype=np.float32)

    total_pads = NC * pp["S"] - N
    bncor = np.zeros((P, 6), np.float32)
    for li, (Aw, av, Bw, cv) in enumerate(
            ((A0, a0, B0, c0), (A1, a1, B1, c1), (A2, a2, B2, c2))):
        ua = np.maximum(np.asarray(av, np.float32), 0.0)
        u_pad = np.maximum(ua @ np.asarray(Bw, np.float32)
                           + np.asarray(cv, np.float32), 0.0)
        bncor[:, 2 * li] = total_pads * u_pad
        bncor[:, 2 * li + 1] = total_pads * u_pad * u_pad

    G_pad = pp["G_pad"]
    in_maps = []
    for c in range(NC):
        ends = pp["end_ids"][c].astype(np.int16)
        endw = ends.reshape(G_pad // 16, 16).T.copy()
        endw = np.tile(endw, (8, 1))
        parw = np.tile(pp["end_par"][c][None, :], (P, 1)).astype(np.float32)
        in_maps.append(dict(
            parw=parw,
            xe=pp["xe"][c],
            m=pp["m"][c],
            idx16=pp["idx16"][c],
            wstk=wstk,
            pvec=pvec.astype(np.float32),
            resets=np.tile(pp["resets"][c][None, :].astype(bf16), (P, 1)),
            endi=endw,
            bncor=bncor,
            ident=ident,
            identf=identf,
            cntv=pp["cnt"][c][None, :].astype(bf16),
            dinv=pp["dinv_nm"][c],
        ))

    trace = bool(os.environ.get("KERNEL_TRACE"))
    res = run_bass_kernel_spmd(nc, in_maps, list(range(NC)), trace=trace)
    kernel.last_exec_ns = res.exec_time_ns
    kernel.last_result = res

    outp = np.zeros((NG, 9 * D), np.float32)
    for c in range(NC):
        oc = res.results[c]["out"].reshape(G_pad, 9 * D)
        Gc = pp["g1"][c] - pp["g0"][c]
        outp[pp["g0"][c]:pp["g1"][c]] = oc[:Gc]
    outp[pp["empty"]] = -np.inf
    return outp


# revision 21
# speedup vs baseline: 1.0777x; 1.0713x over previous
"""Trainium2 Bass kernel v7 for nn_CLGF_GNNDrug (GCN+GIN drug GNN, 8 cores).

Key changes vs v2 baseline (3.18ms):
  - SWDGE dma_gather calls round-robin over 4 SWDGE queues: the single-queue
    ucode stalls on its own ring drain (7.6us/call); 4 queues pipeline prep
    against DMA drain (~2.0us/call, gathers run at ~230GB/s vs ~96GB/s).
  - Sorted-src chunking against HALF tables (2 x 51200 rows) addressed via
    offset-view in_aps (int16 idx is relative to a per-call base row), giving
    ~96% chunk fill vs 67% for the old (tile, quarter) cell scheme. Edge rows
    per layer drop from ~113k to ~80k.
  - Tables AllGathered in 2 chunks/layer (halves), overlapped with compute.
"""
import os
import sys
import types

import numpy as np
import ml_dtypes


def _install_ntff_hook():
    try:
        from antenv.axon_hooks import get_axon_ntff_profile_hook  # noqa: F401
        return
    except ImportError:
        pass
    try:
        from trn_agent_boot.trn_boot import _ntff_profile_via_ctypes
        hook = _ntff_profile_via_ctypes("/opt/axon/libaxon_pjrt.so")
    except Exception:
        hook = None
    mod = types.ModuleType("antenv.axon_hooks")
    mod.get_axon_ntff_profile_hook = lambda: hook
    mod.set_axon_ntff_profile_hook = lambda h: None
    sys.modules["antenv.axon_hooks"] = mod


_install_ntff_hook()

import concourse.bass as bass
import concourse.bacc as bacc
import concourse.mybir as mybir
import concourse.tile as tile
from concourse.bass_utils import run_bass_kernel_spmd

N = 100000
E = 500000
NG = 4000
F_IN = 77
D = 128
BN_EPS = 1e-5
NC = 8
P = 128
GRP = 512
GPT = GRP // P
NH = 2             # half tables per layer
NEG = -1.0e30
MAXCH = 7          # max chunks (x128 idxs) per dma_gather call
WIN = 32768        # int16 idx window (rows) per gather call
NQUEUE = 4         # SWDGE queues

dt = mybir.dt
BF = dt.float16
F32 = dt.float32
bf16 = np.float16


# ============================= host preprocessing =============================

def prep(x, edge_index, batch):
    x = np.asarray(x, np.float32)
    src_all = np.asarray(edge_index[0], np.int64)
    dst_all = np.asarray(edge_index[1], np.int64)
    batch = np.asarray(batch, np.int64)

    gsizes = np.bincount(batch, minlength=NG)
    gstart = np.concatenate([[0], np.cumsum(gsizes)])
    cuts = [0]
    for c in range(1, NC):
        target = c * N // NC
        g = int(np.searchsorted(gstart, target))
        if g > 0 and abs(gstart[g - 1] - target) < abs(gstart[min(g, NG)] - target):
            g -= 1
        g = min(max(g, cuts[-1]), NG)
        cuts.append(g)
    cuts.append(NG)
    g0 = np.array(cuts[:-1]); g1 = np.array(cuts[1:])
    n0 = gstart[g0]; n1 = gstart[g1]
    ncore = (n1 - n0).astype(np.int64)

    S = int(np.ceil(ncore.max() / GRP) * GRP)
    T = S // P
    NGRP = S // GRP
    HR = S // NH           # local rows per half
    NR = NC * HR           # rows per half-table
    Gc = (g1 - g0).astype(np.int64)
    G_pad = int(np.ceil((Gc.max() + 1) / 16) * 16)

    core_of = np.searchsorted(n1, np.arange(N), side="right")
    local = np.arange(N) - n0[core_of]
    shalf = local // HR                    # source half of each node
    srel = core_of * HR + (local % HR)     # row within half-table

    deg = 1.0 + np.bincount(dst_all, minlength=N).astype(np.float64)
    dinv = (1.0 / np.sqrt(deg)).astype(np.float32)

    # regular edges only — self-loops are handled via dedicated per-tile
    # "self chunks" whose rhs is a contiguous load from this core's own
    # emitted rows (ag buffers), not a gather.
    es = src_all
    ed = dst_all

    ecore = core_of[ed]
    dloc = local[ed]
    tl = dloc // P
    fc = (dloc % P).astype(np.int64)
    hh = shalf[es]
    rr = srel[es]

    # per-core edge lists sorted by (cell, table row)
    pc = {}
    for c in range(NC):
        idxs = np.where(ecore == c)[0]
        k_c = tl[idxs] * NH + hh[idxs]
        o = np.lexsort((rr[idxs], k_c))
        idxs = idxs[o]; k_c = k_c[o]
        cnts = np.bincount(k_c, minlength=T * NH)
        cb = np.concatenate([[0], np.cumsum(cnts)])[:-1]
        pc[c] = (idxs, k_c, cnts, cb)

    # shared (SPMD-uniform) segment cuts per cell: cut whenever any core
    # would exceed 128 rows in the segment, or the row span would exceed
    # the int16 gather window.
    nch = np.zeros((T, NH), np.int64)
    cell_cuts = {}
    for t in range(T):
        for h in range(NH):
            k = t * NH + h
            lists = [rr[pc[c][0][pc[c][3][k]:pc[c][3][k] + pc[c][2][k]]]
                     for c in range(NC)]
            ptr = [0] * NC
            cuts = []
            while True:
                rem = [lst[q:] for lst, q in zip(lists, ptr)]
                if all(len(r) == 0 for r in rem):
                    break
                first = min(int(r[0]) for r in rem if len(r))
                cut = first + WIN          # exclusive upper bound
                for r in rem:
                    if len(r) > P:
                        cut = min(cut, int(r[P]))
                assert cut > first, "degenerate segment (>128 equal rows)"
                cuts.append(cut)
                for ci, lst in enumerate(lists):
                    ptr[ci] += int(np.searchsorted(lst[ptr[ci]:], cut))
            cell_cuts[k] = np.array(cuts if cuts else [1], np.int64)
            nch[t, h] = len(cuts)
    MAXJ = max(int(nch.max()), 1)

    # per-core slot assignment + chunk row ranges
    cmin = np.full((T, NH, MAXJ), np.iinfo(np.int64).max, np.int64)
    cmax = np.full((T, NH, MAXJ), -1, np.int64)
    order_all = {}
    for c in range(NC):
        idxs, k_c, cnts, cb = pc[c]
        j = np.zeros(len(idxs), np.int64)
        for k in np.unique(k_c):
            sl = slice(cb[k], cb[k] + cnts[k])
            j[sl] = np.searchsorted(cell_cuts[k], rr[idxs[sl]], side="right")
        key2 = k_c * MAXJ + j
        gs = np.ones(len(idxs), bool)
        gs[1:] = key2[1:] != key2[:-1]
        startidx = np.maximum.accumulate(np.where(gs, np.arange(len(idxs)), 0))
        rank = np.arange(len(idxs)) - startidx
        assert len(rank) == 0 or rank.max() < P
        order_all[c] = (idxs, k_c, j, rank)
        np.minimum.at(cmin.reshape(-1), key2, rr[idxs])
        np.maximum.at(cmax.reshape(-1), key2, rr[idxs])
    empty = cmax.reshape(-1) < 0
    cmin.reshape(-1)[empty] = 0
    cmax.reshape(-1)[empty] = 0

    # call packing per (group, half): greedy by ascending min row.
    # Self chunks (one per tile, rhs loaded contiguously from ag buffers)
    # come first in each group.
    chunkpos = np.full((T, NH, MAXJ), -1, np.int64)
    basearr = []          # per chunk position: call base row
    band_of = []
    calls_by_g = [[] for _ in range(NGRP)]
    selfs_by_g = [[] for _ in range(NGRP)]
    selfpos = np.zeros(T, np.int64)
    pos = 0
    gs0 = []; gcnt = []
    for g in range(NGRP):
        gs0.append(pos)
        for t in range(g * GPT, (g + 1) * GPT):
            k_half = (t * P) // HR
            r0 = t * P - k_half * HR
            selfpos[t] = pos
            selfs_by_g[g].append((pos, t, k_half, r0))
            basearr.append(0)
            band_of.append(t % GPT)
            pos += 1
        for h in range(NH):
            chunks = []
            for t in range(g * GPT, (g + 1) * GPT):
                for j in range(int(nch[t, h])):
                    chunks.append((int(cmin[t, h, j]), int(cmax[t, h, j]), t, j))
            chunks.sort()
            cur = []
            cur_base = 0
            cur_max = 0

            def flush():
                nonlocal pos, cur
                if not cur:
                    return
                pos0 = pos
                for (mn, mx, t, j) in cur:
                    chunkpos[t, h, j] = pos
                    basearr.append(cur_base)
                    band_of.append(t % GPT)
                    pos += 1
                calls_by_g[g].append((h, pos0, len(cur), cur_base))
                cur = []

            for (mn, mx, t, j) in chunks:
                if cur and (len(cur) >= MAXCH or mx - cur_base > WIN - 1):
                    flush()
                if not cur:
                    cur_base = mn
                    cur_max = mx
                cur_max = max(cur_max, mx)
                cur.append((mn, mx, t, j))
                assert cur_max - cur_base <= WIN - 1
            flush()
        gcnt.append(pos - gs0[-1])
    C = pos
    basearr = np.array(basearr, np.int64)
    band_of = np.array(band_of, np.int64)

    # per-tile matmul order: self chunk first, then gather chunks
    tile_chunks = []
    for t in range(T):
        lst = [int(selfpos[t])] + sorted(
            int(p) for p in chunkpos[t].reshape(-1) if p >= 0)
        tile_chunks.append(lst)
    # per-group position count of selfs + half-0 calls (available before the
    # half-1 AllGather)
    n_h0 = []
    for g in range(NGRP):
        n = GPT + sum(nck for (h, p0, nck, b) in calls_by_g[g] if h == 0)
        n_h0.append(n)

    # per-core slot data
    xrow = np.zeros((N, 2 * P), bf16)
    xrow[:, :F_IN] = (dinv[:, None] * x).astype(bf16)
    xrow[:, P:P + F_IN] = x.astype(bf16)

    src16 = np.zeros((NC, C * P), np.int16)
    m = np.zeros((NC, P, C * P), ml_dtypes.float8_e4m3fn)
    xe = np.zeros((NC, P, C * 2 * P), bf16)
    for c in range(NC):
        idxs, k_c, j, rank = order_all[c]
        p = rank
        t_c = k_c // NH
        h_c = k_c % NH
        pos_e = chunkpos[t_c, h_c, j]
        assert (pos_e >= 0).all()
        rel = rr[idxs] - basearr[pos_e]
        if len(rel):
            assert rel.min() >= 0 and rel.max() <= WIN - 1
        src16[c, pos_e * P + p] = rel.astype(np.int16)
        m[c, p, pos_e * P + fc[idxs]] = 1.0
        xe[c].reshape(P, C, 2 * P)[p, pos_e, :] = xrow[es[idxs]]
        # self-loop chunks: node (t*P + p) at slot p of selfpos[t]
        nreal = int(ncore[c])
        loc = np.arange(nreal)
        tt = loc // P
        ps = loc % P
        spos = selfpos[tt]
        m[c, ps, spos * P + ps] = 1.0
        xe[c].reshape(P, C, 2 * P)[ps, spos, :] = xrow[n0[c] + loc]

    # wrapped int16 idx packing (idx n of chunk k -> [n%16, k*8 + n//16])
    idx16 = np.zeros((NC, P, C * 8), np.int16)
    for c in range(NC):
        w = src16[c].reshape(C * 8, 16).T
        idx16[c] = np.tile(w, (8, 1))

    resets = np.zeros((NC, S), np.float32)
    end_ids = np.zeros((NC, G_pad), np.int64)
    end_par = np.zeros((NC, G_pad), np.float32)
    for c in range(NC):
        gs = gstart[g0[c]:g1[c] + 1] - n0[c]
        starts = gs[:-1]; ends = gs[1:] - 1
        ne = gsizes[g0[c]:g1[c]] > 0
        resets[c, starts[ne]] = NEG
        if ncore[c] < S:
            resets[c, ncore[c]] = NEG
        end_ids[c, :g1[c] - g0[c]][ne] = ends[ne] // 2
        end_par[c, :g1[c] - g0[c]][ne] = (ends[ne] % 2).astype(np.float32)

    cntdeg = np.zeros((NC, S), np.float32)
    dinv_nm = np.ones((NC, P, T), np.float32)
    for c in range(NC):
        cntdeg[c, :ncore[c]] = deg[n0[c]:n1[c]].astype(np.float32)
        dv = dinv[n0[c]:n1[c]]
        dpad = np.ones(S, np.float32)
        dpad[:ncore[c]] = dv
        dinv_nm[c] = dpad.reshape(T, P).T

    return dict(
        S=S, T=T, NGRP=NGRP, HR=HR, NR=NR, C=C, G_pad=G_pad,
        calls_by_g=calls_by_g, selfs_by_g=selfs_by_g,
        gs0=gs0, gcnt=gcnt, band_of=band_of,
        tile_chunks=tile_chunks, n_h0=n_h0,
        g0=g0, g1=g1, n0=n0, n1=n1, ncore=ncore,
        src16=src16, m=m, xe=xe, idx16=idx16,
        resets=resets, end_ids=end_ids, end_par=end_par,
        empty=(gsizes == 0), cnt=cntdeg, dinv_nm=dinv_nm,
    )


# ============================= device program =============================

def build_program(meta, debug=False):
    S = meta["S"]; T = meta["T"]; NGRP = meta["NGRP"]
    HR = meta["HR"]; NR = meta["NR"]; C = meta["C"]; G_pad = meta["G_pad"]
    calls_by_g = meta["calls_by_g"]
    selfs_by_g = meta["selfs_by_g"]
    gs0 = meta["gs0"]; gcnt = meta["gcnt"]
    tile_chunks = meta["tile_chunks"]; n_h0 = meta["n_h0"]

    nc = bacc.Bacc("TRN2", target_bir_lowering=False, num_swdge_queues=NQUEUE)
    AluOp = mybir.AluOpType
    Act = mybir.ActivationFunctionType

    xe_d = nc.dram_tensor("xe", [P, C * 2 * P], BF, kind="ExternalInput")
    m_d = nc.dram_tensor("m", [P, C * P], dt.float8e4, kind="ExternalInput")
    idx_d = nc.dram_tensor("idx16", [P, C * 8], dt.int16, kind="ExternalInput")
    wstk = nc.dram_tensor("wstk", [8 * P, D], BF, kind="ExternalInput")
    pvec = nc.dram_tensor("pvec", [P, 14], F32, kind="ExternalInput")
    rst_d = nc.dram_tensor("resets", [P, S], BF, kind="ExternalInput")
    endi = nc.dram_tensor("endi", [P, G_pad // 16], dt.int16, kind="ExternalInput")
    bncor = nc.dram_tensor("bncor", [P, 6], F32, kind="ExternalInput")
    ident = nc.dram_tensor("ident", [P, P], BF, kind="ExternalInput")
    identf = nc.dram_tensor("identf", [P, P], F32, kind="ExternalInput")
    cntv = nc.dram_tensor("cntv", [1, S], BF, kind="ExternalInput")
    dinv_d = nc.dram_tensor("dinv", [P, T], F32, kind="ExternalInput")
    parw_d = nc.dram_tensor("parw", [P, G_pad], F32, kind="ExternalInput")

    out = nc.dram_tensor("out", [G_pad * 9, D], F32, kind="ExternalOutput")

    ag2h = [nc.dram_tensor(f"ag2_{k}", [HR, 2 * D], BF) for k in range(NH)]
    tab2h = [nc.dram_tensor(f"tab2_{k}", [NR, 2 * D], BF, addr_space="Shared")
             for k in range(NH)]
    ag3h = [nc.dram_tensor(f"ag3_{k}", [HR, D], BF) for k in range(NH)]
    tab3h = [nc.dram_tensor(f"tab3_{k}", [NR, D], BF, addr_space="Shared")
             for k in range(NH)]
    bn_in = [nc.dram_tensor(f"bn{i}_in", [P, 2], F32) for i in range(3)]
    bn_out = [nc.dram_tensor(f"bn{i}_out", [P, 2], F32, addr_space="Shared")
              for i in range(3)]
    SL = {}
    slkind = dict(kind="ExternalOutput") if debug else {}
    for nme in ("xg1", "xg2", "u0", "u1", "u2"):
        SL[nme] = nc.dram_tensor(f"sl_{nme}", [P, S], BF, **slkind)
    if debug:
        dbg_scan = nc.dram_tensor("dbg_scan", [P, 2 * S], BF,
                                  kind="ExternalOutput")
        dbg_ext = nc.dram_tensor("dbg_ext", [P, 4 * G_pad], BF,
                                 kind="ExternalOutput")
        dbg_pool = nc.dram_tensor("dbg_pool", [P, G_pad * 9], F32,
                                  kind="ExternalOutput")

    RG = [list(range(NC))]
    # AllGather chunk trigger group: half k ready after group trig[k]
    trig = {}
    for k in range(NH):
        trig[((k + 1) * HR - 1) // GRP] = k

    qctr = [0]

    def next_q():
        q = qctr[0] % NQUEUE
        qctr[0] += 1
        return q

    with tile.TileContext(nc) as tc:
        with (
            tc.tile_pool(name="cst", bufs=1) as cst,
            tc.tile_pool(name="scn", bufs=3) as scnp,
            tc.tile_pool(name="gat", bufs=2) as gat,
            tc.tile_pool(name="mbuf", bufs=2) as mbp,
            tc.tile_pool(name="work", bufs=2) as wkp,
            tc.tile_pool(name="one", bufs=1) as onep,
            tc.tile_pool(name="ps2b", bufs=4, space="PSUM") as psa_p,
            tc.tile_pool(name="psd", bufs=1, space="PSUM") as psd,
            tc.tile_pool(name="pst", bufs=2, space="PSUM") as pst,
        ):
            # ---------------- constants ----------------
            w_sb = cst.tile([P, 8 * D], BF)
            for i in range(8):
                nc.sync.dma_start(out=w_sb[:, i * D:(i + 1) * D],
                                  in_=wstk[i * P:(i + 1) * P, :])
            pv = cst.tile([P, 14], F32)
            nc.sync.dma_start(out=pv[:], in_=pvec[:, :])
            endi_sb = cst.tile([P, G_pad // 16], dt.int16)
            nc.sync.dma_start(out=endi_sb[:], in_=endi[:, :])
            idn = cst.tile([P, P], BF)
            nc.sync.dma_start(out=idn[:], in_=ident[:, :])
            idnf = cst.tile([P, P], F32)
            nc.sync.dma_start(out=idnf[:], in_=identf[:, :])
            bnc = cst.tile([P, 6], F32)
            nc.sync.dma_start(out=bnc[:], in_=bncor[:, :])
            dinv_sb = cst.tile([P, T], F32)
            nc.sync.dma_start(out=dinv_sb[:], in_=dinv_d[:, :])
            parw_sb = cst.tile([P, G_pad], F32)
            nc.sync.dma_start(out=parw_sb[:], in_=parw_d[:, :])
            aeff_sb = cst.tile([P, 2 * D], BF)
            atv_row = cst.tile([1, 2 * P], BF)

            W1, W2 = w_sb[:, 0:D], w_sb[:, D:2 * D]
            A = [w_sb[:, (2 + 2 * i) * D:(3 + 2 * i) * D] for i in range(3)]
            B = [w_sb[:, (3 + 2 * i) * D:(4 + 2 * i) * D] for i in range(3)]
            b1c, b2c = pv[:, 0:1], pv[:, 1:2]
            ac = [pv[:, 2 + 4 * i:3 + 4 * i] for i in range(3)]
            cc = [pv[:, 3 + 4 * i:4 + 4 * i] for i in range(3)]
            gcl = [pv[:, 4 + 4 * i:5 + 4 * i] for i in range(3)]
            bec = [pv[:, 5 + 4 * i:6 + 4 * i] for i in range(3)]

            pooled = cst.tile([P, G_pad * 9], F32)
            stats = cst.tile([P, 3 * 2 * NGRP], F32)
            sf = cst.tile([P, 8], F32)
            bnreg = cst.tile([P, 9], F32)

            scanbufs = {}
            prevs = {}

            def load_rst(g):
                rt = wkp.tile([P, GRP], BF, tag="rstg", name=f"rst_{g}")
                nc.sync.dma_start(out=rt[:],
                                  in_=rst_d[:, g * GRP:(g + 1) * GRP])
                return rt

            def scan_g(comp, g, data1, rt):
                if comp not in scanbufs:
                    scanbufs[comp] = scnp.tile([P, S], BF, tag="scan",
                                               name=f"scan_c{comp}")
                    prevs[comp] = 0.0
                sb = scanbufs[comp]
                nc.vector.tensor_tensor_scan(
                    out=sb[:, g * GRP:(g + 1) * GRP],
                    data0=rt[:],
                    data1=data1,
                    initial=prevs[comp],
                    op0=AluOp.add,
                    op1=AluOp.max)
                prevs[comp] = sb[:, (g + 1) * GRP - 1:(g + 1) * GRP]

            def extract(comp):
                # bf16 ap_gather needs d=2: gather the PAIR containing the
                # graph-end column, then select the half via the parity mask.
                sb = scanbufs[comp]
                ext = wkp.tile([P, G_pad * 2], BF, tag="nm",
                                name=f"ext{comp}")
                nc.gpsimd.ap_gather(
                    out_ap=ext[:].rearrange("p (g o) -> p g o", o=2),
                    in_ap=sb[:].rearrange("p (s o) -> p s o", o=2),
                    idxs_ap=endi_sb[:], channels=P, num_elems=S // 2, d=2,
                    num_idxs=G_pad)
                pv_ = pooled[:, comp::9]
                nc.vector.tensor_tensor(out=pv_, in0=ext[:, 1::2],
                                        in1=ext[:, 0::2], op=AluOp.subtract)
                nc.vector.tensor_tensor(out=pv_, in0=pv_, in1=parw_sb[:],
                                        op=AluOp.mult)
                nc.vector.tensor_tensor(out=pv_, in0=pv_, in1=ext[:, 0::2],
                                        op=AluOp.add)
                if debug and comp in (0, 4):
                    hf = comp // 4
                    nc.sync.dma_start(out=dbg_scan[:, hf * S:(hf + 1) * S],
                                      in_=sb[:])
                    nc.sync.dma_start(
                        out=dbg_ext[:, hf * 2 * G_pad:(hf + 1) * 2 * G_pad],
                        in_=ext[:])

            # PSUM bands live at 2KB (bank) strides: a matmul with start=True
            # zeroes its entire PSUM bank, so two open accumulations must
            # never share a bank.
            BANDW = 512  # f32 elems per band slot = one 2KB bank

            def agg_tiles(g, rhs0, rhs1, split, elem, halves):
                """Per-tile band accumulation -> feature-major agg tiles.

                Phase A issues every tile's half-0 (selfs + early-table)
                matmuls first, so the PE keeps working while the half-1
                AllGather is still in flight; phase A2 adds the half-1
                matmuls; phase B drains the bands (copies + transposes).
                Bands rotate through 4 PSUM banks."""
                s0 = gs0[g]
                aggs = {}
                for nme, _ in halves:
                    aggs[nme] = wkp.tile([P, GRP], BF, tag=f"agg{nme}",
                                         name=f"agg_{nme}_{g}")
                bands = []
                for ti in range(GPT):
                    t = g * GPT + ti
                    lst = tile_chunks[t]
                    lst0 = [p for p in lst if p - s0 < split]
                    lst1 = [p for p in lst if p - s0 >= split]
                    band = psa_p.tile([P, BANDW], F32, tag="psa",
                                      name=f"psa_{elem}_{g}_{ti}")
                    bands.append((band, lst0, lst1))
                    for i, pos in enumerate(lst0):
                        off = pos - s0
                        nc.tensor.matmul(
                            band[:, 0:elem],
                            m_t[:, off * P:(off + 1) * P],
                            rhs0[:, off * elem:(off + 1) * elem],
                            start=(i == 0),
                            stop=(not lst1 and i == len(lst0) - 1))
                for ti in range(GPT):
                    band, lst0, lst1 = bands[ti]
                    for i, pos in enumerate(lst1):
                        off = pos - s0
                        o2 = off - split
                        nc.tensor.matmul(
                            band[:, 0:elem],
                            m_t[:, off * P:(off + 1) * P],
                            rhs1[:, o2 * elem:(o2 + 1) * elem],
                            start=False, stop=(i == len(lst1) - 1))
                for ti in range(GPT):
                    t = g * GPT + ti
                    band = bands[ti][0]
                    col = dinv_sb[:, t:t + 1]
                    nm = wkp.tile([P, len(halves) * P], BF, tag="nm",
                                  name=f"nm_{elem}_{g}_{ti}")
                    for hi, (nme, scaled) in enumerate(halves):
                        src = band[:, hi * P:(hi + 1) * P]
                        dst = nm[:, hi * P:(hi + 1) * P]
                        if scaled:
                            nc.scalar.activation(dst, src, Act.Identity,
                                                 scale=col)
                        elif hi % 2 == 0:
                            nc.scalar.copy(dst, src)
                        else:
                            nc.vector.tensor_copy(dst, src)
                    for hi, (nme, _) in enumerate(halves):
                        ptile = pst.tile([P, P], BF, tag="tp",
                                         name=f"tp_{nme}_{g}_{ti}")
                        nc.tensor.transpose(
                            ptile[:], nm[:, hi * P:(hi + 1) * P], idn[:])
                        if ti % 2 == 0:
                            nc.vector.tensor_copy(
                                aggs[nme][:, ti * P:(ti + 1) * P], ptile[:])
                        else:
                            nc.scalar.copy(
                                aggs[nme][:, ti * P:(ti + 1) * P], ptile[:])
                return aggs

            def emit_nm(g, o_tile, dest_list, col0, width, scale_dinv):
                """Transpose feature-major dense output to node-major rows and
                DMA into the collective-input tensors (split at HR bounds)."""
                em = wkp.tile([P, GPT * P], BF, tag="em", name=f"em_{g}_{col0}")
                for t in range(GPT):
                    ptile = pst.tile([P, P], BF, tag="tp",
                                     name=f"em_tp_{g}_{col0}_{t}")
                    nc.tensor.transpose(ptile[:], o_tile[:, t * P:(t + 1) * P],
                                        idn[:])
                    if scale_dinv:
                        nc.scalar.activation(
                            em[:, t * P:(t + 1) * P], ptile[:], Act.Identity,
                            scale=dinv_sb[:, g * GPT + t:g * GPT + t + 1])
                    else:
                        nc.scalar.copy(em[:, t * P:(t + 1) * P], ptile[:])
                # DMA node-major: rows g*GRP + t*128 + p
                r0 = g * GRP
                t0 = 0
                while t0 < GPT:
                    k = (r0 + t0 * P) // HR
                    tmax = min(GPT, ((k + 1) * HR - r0) // P)
                    nt = tmax - t0
                    dest = dest_list[k]
                    rr_ = r0 + t0 * P - k * HR
                    dst_ap = dest[rr_:rr_ + nt * P, col0:col0 + width]
                    dst_ap = dst_ap.rearrange("(t p) f -> p t f", p=P)
                    src_ap = em[:, t0 * P:(t0 + nt) * P]
                    src_ap = src_ap.rearrange("p (t f) -> p t f", f=P)
                    nc.sync.dma_start(out=dst_ap, in_=src_ap)
                    t0 = tmax

            def dense_gcn(g, aggbuf, W, bcol, sl_dest):
                ps = psd.tile([P, GRP], F32, tag="zd")
                nc.tensor.matmul(ps[:], W, aggbuf[:], start=True, stop=True)
                o = wkp.tile([P, GRP], BF, tag="obf", name=f"ogcn_{g}")
                nc.scalar.activation(o[:], ps[:], Act.Relu, bias=bcol)
                nc.sync.dma_start(
                    out=sl_dest[:, g * GRP:(g + 1) * GRP], in_=o[:])
                return o

            def dense_gin(g, aggbuf, li, sl_dest):
                ps1 = psd.tile([P, GRP], F32, tag="zd")
                if li == 0:
                    nc.tensor.matmul(ps1[:], A[0], aggbuf[:],
                                     start=True, stop=True)
                else:
                    Aeff = aeff_sb[:, (li - 1) * D:li * D]
                    nc.tensor.matmul(ps1[:], Aeff, aggbuf[:],
                                     start=True, stop=False)
                    cg = wkp.tile([1, GRP], BF, tag="cntg",
                                  name=f"cg{li}_{g}")
                    nc.sync.dma_start(
                        out=cg[:], in_=cntv[0:1, g * GRP:(g + 1) * GRP])
                    nc.tensor.matmul(
                        ps1[:], atv_row[0:1, (li - 1) * P:li * P],
                        cg[0:1, :], start=False, stop=True)
                ua = wkp.tile([P, GRP], BF, tag="ua", name=f"ua{li}_{g}")
                nc.scalar.activation(ua[:], ps1[:], Act.Relu, bias=ac[li])
                ps2 = psd.tile([P, GRP], F32, tag="zd2")
                nc.tensor.matmul(ps2[:], B[li], ua[:], start=True, stop=True)
                o32 = wkp.tile([P, GRP], F32, tag="o32", name=f"og{li}_{g}")
                base = li * 2 * NGRP
                nc.scalar.activation(o32[:], ps2[:], Act.Relu, bias=cc[li],
                                     accum_out=stats[:, base + 2 * g:
                                                     base + 2 * g + 1])
                sq = onep.tile([P, GRP], F32, tag="sq", name=f"sq{li}_{g}")
                nc.vector.tensor_tensor(out=sq[:], in0=o32[:], in1=o32[:],
                                        op=AluOp.mult)
                nc.vector.reduce_sum(
                    stats[:, base + 2 * g + 1:base + 2 * g + 2], sq[:],
                    axis=mybir.AxisListType.X)
                obf = wkp.tile([P, GRP], BF, tag="obf", name=f"ogb{li}_{g}")
                nc.vector.tensor_copy(obf[:], o32[:])
                nc.sync.dma_start(
                    out=sl_dest[:, g * GRP:(g + 1) * GRP], in_=obf[:])
                return o32, obf

            def bn_finalize(li):
                base = li * 2 * NGRP
                nc.vector.reduce_sum(sf[:, 0:1], stats[:, base:base + 2 * NGRP:2],
                                     axis=mybir.AxisListType.X)
                nc.vector.reduce_sum(sf[:, 1:2],
                                     stats[:, base + 1:base + 2 * NGRP:2],
                                     axis=mybir.AxisListType.X)
                nc.vector.tensor_tensor(out=sf[:, 0:2], in0=sf[:, 0:2],
                                        in1=bnc[:, 2 * li:2 * li + 2],
                                        op=AluOp.subtract)
                nc.sync.dma_start(out=bn_in[li][:, :], in_=sf[:, 0:2])
                nc.gpsimd.collective_compute(
                    "AllReduce", AluOp.add, replica_groups=RG,
                    ins=[bn_in[li][:, :]], outs=[bn_out[li][:, :]])
                nc.sync.dma_start(out=sf[:, 2:4], in_=bn_out[li][:, :])
                nc.vector.tensor_scalar(out=sf[:, 4:5], in0=sf[:, 2:3],
                                        scalar1=1.0 / N, scalar2=None,
                                        op0=AluOp.mult)
                nc.vector.tensor_scalar(out=sf[:, 5:6], in0=sf[:, 3:4],
                                        scalar1=1.0 / N, scalar2=None,
                                        op0=AluOp.mult)
                nc.vector.tensor_tensor(out=sf[:, 6:7], in0=sf[:, 4:5],
                                        in1=sf[:, 4:5], op=AluOp.mult)
                nc.vector.tensor_tensor(out=sf[:, 5:6], in0=sf[:, 5:6],
                                        in1=sf[:, 6:7], op=AluOp.subtract)
                nc.vector.tensor_scalar(out=sf[:, 5:6], in0=sf[:, 5:6],
                                        scalar1=BN_EPS, scalar2=None,
                                        op0=AluOp.add)
                nc.scalar.activation(sf[:, 5:6], sf[:, 5:6], Act.Sqrt)
                nc.vector.reciprocal(sf[:, 6:7], sf[:, 5:6])
                nc.vector.tensor_tensor(out=sf[:, 6:7], in0=sf[:, 6:7],
                                        in1=gcl[li], op=AluOp.mult)
                nc.vector.tensor_tensor(out=sf[:, 7:8], in0=sf[:, 4:5],
                                        in1=sf[:, 6:7], op=AluOp.mult)
                nc.vector.tensor_tensor(out=sf[:, 7:8], in0=bec[li],
                                        in1=sf[:, 7:8], op=AluOp.subtract)
                scol = bnreg[:, 3 * li:3 * li + 1]
                tcol = bnreg[:, 3 * li + 1:3 * li + 2]
                rcol = bnreg[:, 3 * li + 2:3 * li + 3]
                nc.vector.tensor_copy(scol, sf[:, 6:7])
                nc.vector.tensor_copy(tcol, sf[:, 7:8])
                nc.vector.reciprocal(sf[:, 0:1], sf[:, 6:7])
                nc.vector.tensor_tensor(out=rcol, in0=sf[:, 7:8],
                                        in1=sf[:, 0:1], op=AluOp.mult)
                if li < 2:
                    nc.vector.tensor_scalar(
                        out=aeff_sb[:, li * D:(li + 1) * D], in0=A[li + 1],
                        scalar1=sf[:, 6:7], scalar2=None, op0=AluOp.mult)
                    tbf = wkp.tile([P, 1], BF, tag="nm1", name=f"tbf{li}")
                    nc.vector.tensor_copy(tbf[:], sf[:, 7:8])
                    pv1 = pst.tile([P, 1], F32, tag="tp", name=f"atv{li}")
                    nc.tensor.matmul(pv1[:], A[li + 1], tbf[:],
                                     start=True, stop=True)
                    atc = wkp.tile([P, 1], BF, tag="nm1", name=f"atc{li}")
                    nc.vector.tensor_copy(atc[:], pv1[:])
                    pv2 = pst.tile([1, P], BF, tag="tp", name=f"atr{li}")
                    nc.tensor.transpose(pv2[:], atc[:], idn[:])
                    nc.vector.tensor_copy(atv_row[0:1, li * P:(li + 1) * P],
                                          pv2[:])

            def issue_h0(g, ag_list, tabs, elem):
                split = n_h0[g]
                gt0 = gat.tile([P, split * elem], BF, tag="g0",
                               name=f"g0{elem}_{g}")
                s0 = gs0[g]
                # self chunks: contiguous load of this core's own emitted rows
                for (pos, t, kh, r0) in selfs_by_g[g]:
                    off = pos - s0
                    nc.sync.dma_start(
                        out=gt0[:, off * elem:(off + 1) * elem],
                        in_=ag_list[kh][r0:r0 + P, :])
                idx_t = wkp.tile([P, gcnt[g] * 8], dt.int16, tag="idxg2",
                                 name=f"idx_{elem}_{g}")
                nc.sync.dma_start(
                    out=idx_t[:], in_=idx_d[:, s0 * 8:(s0 + gcnt[g]) * 8])
                for (h, pos0, nck, cbase) in calls_by_g[g]:
                    if h != 0:
                        continue
                    o0 = pos0 - s0
                    n = nck * P
                    W = min(WIN, NR - cbase)
                    nview = gt0[:, o0 * elem:(o0 + nck) * elem]
                    nc.gpsimd.dma_gather(
                        nview.rearrange("p (j e) -> p j e", e=elem),
                        tabs[0][cbase:cbase + W, :],
                        idx_t[:, o0 * 8:(o0 + nck) * 8],
                        n, n, elem, queue_num=next_q(),
                    )
                return gt0, idx_t

            def issue_h1(g, tabs, elem, idx_t):
                split = n_h0[g]
                gt1 = gat.tile([P, (gcnt[g] - split) * elem], BF, tag="g1",
                               name=f"g1{elem}_{g}")
                s0 = gs0[g]
                for (h, pos0, nck, cbase) in calls_by_g[g]:
                    if h != 1:
                        continue
                    o0 = pos0 - s0
                    o1 = o0 - split
                    n = nck * P
                    W = min(WIN, NR - cbase)
                    nview = gt1[:, o1 * elem:(o1 + nck) * elem]
                    nc.gpsimd.dma_gather(
                        nview.rearrange("p (j e) -> p j e", e=elem),
                        tabs[1][cbase:cbase + W, :],
                        idx_t[:, o0 * 8:(o0 + nck) * 8],
                        n, n, elem, queue_num=next_q(),
                    )
                return gt1

            def load_m(g):
                mt = mbp.tile([P, gcnt[g] * P], dt.float8e4, tag="m",
                                name=f"m_{g}")
                nc.sync.dma_start(
                    out=mt[:], in_=m_d[:, gs0[g] * P:(gs0[g] + gcnt[g]) * P])
                return mt

            def load_sl(nme, g, tag):
                st = wkp.tile([P, GRP], BF, tag=tag, name=f"ld_{nme}_{g}")
                nc.sync.dma_start(out=st[:],
                                  in_=SL[nme][:, g * GRP:(g + 1) * GRP])
                return st

            # ================= LAYER 1 =================
            for g in range(NGRP):
                xe_t = gat.tile([P, gcnt[g] * 2 * P], BF, tag="g0",
                                name=f"xe_{g}")
                nc.sync.dma_start(
                    out=xe_t[:],
                    in_=xe_d[:, gs0[g] * 2 * P:(gs0[g] + gcnt[g]) * 2 * P])
                m_t = load_m(g)
                aggs = agg_tiles(g, xe_t, xe_t, gcnt[g], 2 * P,
                                 [("gc", True), ("gi", False)])
                rt = load_rst(g)
                o_xg1 = dense_gcn(g, aggs["gc"], W1, b1c, SL["xg1"])
                scan_g(0, g, o_xg1[:], rt)
                emit_nm(g, o_xg1, ag2h, 0, D, True)
                o_u0, o_u0b = dense_gin(g, aggs["gi"], 0, SL["u0"])
                scan_g(4, g, o_u0[:], rt)
                emit_nm(g, o_u0b, ag2h, D, D, False)
                if g in trig and trig[g] < NH - 1:
                    k = trig[g]
                    nc.gpsimd.collective_compute(
                        "AllGather", AluOp.bypass, replica_groups=RG,
                        ins=[ag2h[k][:, :]], outs=[tab2h[k][:, :]])

            nc.gpsimd.collective_compute(
                "AllGather", AluOp.bypass, replica_groups=RG,
                ins=[ag2h[NH - 1][:, :]], outs=[tab2h[NH - 1][:, :]])
            bn_finalize(0)
            extract(0)
            extract(4)

            # ================= LAYER 2 =================
            pend2 = {0: issue_h0(0, ag2h, tab2h, 2 * P)}
            for g in range(NGRP):
                if g + 1 < NGRP:
                    pend2[g + 1] = issue_h0(g + 1, ag2h, tab2h, 2 * P)
                gt0, idx_t = pend2.pop(g)
                gt1 = issue_h1(g, tab2h, 2 * P, idx_t)
                m_t = load_m(g)
                aggs = agg_tiles(g, gt0, gt1, n_h0[g], 2 * P,
                                 [("gc", True), ("gi", False)])
                rt = load_rst(g)
                o_xg2 = dense_gcn(g, aggs["gc"], W2, b2c, SL["xg2"])
                scan_g(1, g, o_xg2[:], rt)
                o_u1, o_u1b = dense_gin(g, aggs["gi"], 1, SL["u1"])
                scan_g(5, g, o_u1[:], rt)
                emit_nm(g, o_u1b, ag3h, 0, D, False)
                if g in trig and trig[g] < NH - 1:
                    k = trig[g]
                    nc.gpsimd.collective_compute(
                        "AllGather", AluOp.bypass, replica_groups=RG,
                        ins=[ag3h[k][:, :]], outs=[tab3h[k][:, :]])

            nc.gpsimd.collective_compute(
                "AllGather", AluOp.bypass, replica_groups=RG,
                ins=[ag3h[NH - 1][:, :]], outs=[tab3h[NH - 1][:, :]])
            bn_finalize(1)
            extract(1)
            extract(5)

            s1c = bnreg[:, 0:1]; t1c = bnreg[:, 1:2]; r1c = bnreg[:, 2:3]
            s2c = bnreg[:, 3:4]; t2c = bnreg[:, 4:5]; r2c = bnreg[:, 5:6]
            s3c = bnreg[:, 6:7]; t3c = bnreg[:, 7:8]; r3c = bnreg[:, 8:9]

            # comps 2/3 scans: only need xg1/xg2 — run while tab3h gathers
            # wait on the half-1 AllGather
            for g in range(NGRP):
                rt = load_rst(g)
                xg1t = load_sl("xg1", g, "ldA")
                xg2t = load_sl("xg2", g, "ldB")
                tsum = wkp.tile([P, GRP], BF, tag="tt1", name=f"c2in_{g}")
                nc.vector.tensor_tensor(out=tsum[:], in0=xg1t[:], in1=xg2t[:],
                                        op=AluOp.add)
                scan_g(2, g, tsum[:], rt)
                tprd = wkp.tile([P, GRP], BF, tag="tt2", name=f"c3in_{g}")
                nc.vector.tensor_tensor(out=tprd[:], in0=xg1t[:], in1=xg2t[:],
                                        op=AluOp.mult)
                scan_g(3, g, tprd[:], rt)

            # ================= LAYER 3 =================
            pend3 = {0: issue_h0(0, ag3h, tab3h, P)}
            for g in range(NGRP):
                if g + 1 < NGRP:
                    pend3[g + 1] = issue_h0(g + 1, ag3h, tab3h, P)
                gt0, idx_t = pend3.pop(g)
                gt1 = issue_h1(g, tab3h, P, idx_t)
                m_t = load_m(g)
                aggs = agg_tiles(g, gt0, gt1, n_h0[g], P, [("gi", False)])
                rt = load_rst(g)
                o_u2, _ = dense_gin(g, aggs["gi"], 2, SL["u2"])
                scan_g(6, g, o_u2[:], rt)

            bn_finalize(2)
            extract(2)
            extract(3)
            extract(6)

            # ================= TAIL: comps 7, 8 =================
            for g in range(NGRP):
                u0t = wkp.tile([P, GRP], BF, tag="ldA", name=f"lu0t_{g}")
                nc.sync.dma_start(out=u0t[:],
                                  in_=SL["u0"][:, g * GRP:(g + 1) * GRP])
                u1t = wkp.tile([P, GRP], BF, tag="ldB", name=f"lu1t_{g}")
                nc.sync.dma_start(out=u1t[:],
                                  in_=SL["u1"][:, g * GRP:(g + 1) * GRP])
                u2t = wkp.tile([P, GRP], BF, tag="ldC", name=f"lu2_{g}")
                nc.sync.dma_start(out=u2t[:],
                                  in_=SL["u2"][:, g * GRP:(g + 1) * GRP])
                rt = load_rst(g)
                h1 = wkp.tile([P, GRP], BF, tag="tt3", name=f"h1_{g}")
                nc.scalar.activation(h1[:], u0t[:], Act.Identity, bias=t1c,
                                     scale=s1c)
                h2 = wkp.tile([P, GRP], BF, tag="tt4", name=f"h2_{g}")
                nc.scalar.activation(h2[:], u1t[:], Act.Identity, bias=t2c,
                                     scale=s2c)
                h3 = wkp.tile([P, GRP], BF, tag="tt5", name=f"h3_{g}")
                nc.scalar.activation(h3[:], u2t[:], Act.Identity, bias=t3c,
                                     scale=s3c)
                z3 = wkp.tile([P, GRP], BF, tag="tt6", name=f"z3_{g}")
                nc.vector.tensor_tensor(out=z3[:], in0=h1[:], in1=h2[:],
                                        op=AluOp.add)
                nc.vector.tensor_tensor(out=z3[:], in0=z3[:], in1=h3[:],
                                        op=AluOp.add)
                scan_g(7, g, z3[:], rt)
                w3 = wkp.tile([P, GRP], BF, tag="tt7", name=f"w3_{g}")
                nc.gpsimd.tensor_tensor(out=w3[:], in0=h1[:], in1=h2[:],
                                        op=AluOp.mult)
                nc.gpsimd.tensor_tensor(out=w3[:], in0=w3[:], in1=h3[:],
                                        op=AluOp.mult)
                scan_g(8, g, w3[:], rt)

            extract(7)
            extract(8)

            # pooled-domain BN affine fixups
            for comp, sc, tc_ in ((4, s1c, t1c), (5, s2c, t2c), (6, s3c, t3c)):
                nc.scalar.activation(pooled[:, comp::9], pooled[:, comp::9],
                                     Act.Identity, bias=tc_, scale=sc)

            if debug:
                nc.sync.dma_start(out=dbg_pool[:, :], in_=pooled[:])

            # final transpose-out
            NPT = (G_pad * 9 + P - 1) // P
            for t in range(NPT):
                c0 = t * P
                w = min(P, G_pad * 9 - c0)
                ptile = pst.tile([P, P], F32, tag="tp", name=f"po_{t}")
                nc.tensor.transpose(ptile[:w, :], pooled[:, c0:c0 + w], idnf[:])
                nmo = wkp.tile([P, P], F32, tag="obf", name=f"pon_{t}")
                nc.vector.tensor_copy(nmo[:w, :], ptile[:w, :])
                nc.sync.dma_start(out=out[c0:c0 + w, :], in_=nmo[:w, :])

    nc.finalize()
    return nc


# ============================= top-level kernel =============================

_CACHE = {}


def kernel(x, edge_index, batch, W1, b1, W2, b2,
           A0, a0, B0, c0, g0, be0,
           A1, a1, B1, c1, g1, be1,
           A2, a2, B2, c2, g2, be2):
    pp = prep(x, edge_index, batch)

    debug = bool(os.environ.get("KERNEL_DEBUG"))
    key = (pp["S"], pp["C"], pp["G_pad"], debug,
           repr(pp["calls_by_g"]), pp["band_of"].tobytes())
    if key not in _CACHE:
        _CACHE[key] = build_program(pp, debug=debug)
    nc = _CACHE[key]

    def pad_w(W):
        Wp = np.zeros((P, D), np.float32)
        W = np.asarray(W, np.float32)
        Wp[:W.shape[0]] = W
        return Wp

    wstk = np.concatenate([pad_w(W1), pad_w(W2), pad_w(A0), pad_w(B0),
                           pad_w(A1), pad_w(B1), pad_w(A2), pad_w(B2)],
                          axis=0).astype(bf16)
    pvec = np.stack([np.asarray(v, np.float32) for v in
                     (b1, b2, a0, c0, g0, be0, a1, c1, g1, be1,
                      a2, c2, g2, be2)], axis=1)
    ident = np.eye(P, dtype=bf16)
    identf = np.eye(P, dtype=np.float32)

    total_pads = NC * pp["S"] - N
    bncor = np.zeros((P, 6), np.float32)
    for li, (Aw, av, Bw, cv) in enumerate(
            ((A0, a0, B0, c0), (A1, a1, B1, c1), (A2, a2, B2, c2))):
        ua = np.maximum(np.asarray(av, np.float32), 0.0)
        u_pad = np.maximum(ua @ np.asarray(Bw, np.float32)
                           + np.asarray(cv, np.float32), 0.0)
        bncor[:, 2 * li] = total_pads * u_pad
        bncor[:, 2 * li + 1] = total_pads * u_pad * u_pad

    G_pad = pp["G_pad"]
    in_maps = []
    for c in range(NC):
        ends = pp["end_ids"][c].astype(np.int16)
        endw = ends.reshape(G_pad // 16, 16).T.copy()
        endw = np.tile(endw, (8, 1))
        parw = np.tile(pp["end_par"][c][None, :], (P, 1)).astype(np.float32)
        in_maps.append(dict(
            parw=parw,
            xe=pp["xe"][c],
            m=pp["m"][c],
            idx16=pp["idx16"][c],
            wstk=wstk,
            pvec=pvec.astype(np.float32),
            resets=np.tile(pp["resets"][c][None, :].astype(bf16), (P, 1)),
            endi=endw,
            bncor=bncor,
            ident=ident,
            identf=identf,
            cntv=pp["cnt"][c][None, :].astype(bf16),
            dinv=pp["dinv_nm"][c],
        ))

    trace = bool(os.environ.get("KERNEL_TRACE"))
    res = run_bass_kernel_spmd(nc, in_maps, list(range(NC)), trace=trace)
    kernel.last_exec_ns = res.exec_time_ns
    kernel.last_result = res

    outp = np.zeros((NG, 9 * D), np.float32)
    for c in range(NC):
        oc = res.results[c]["out"].reshape(G_pad, 9 * D)
        Gc = pp["g1"][c] - pp["g0"][c]
        outp[pp["g0"][c]:pp["g1"][c]] = oc[:Gc]
    outp[pp["empty"]] = -np.inf
    return outp
